# revision 28
# baseline (speedup 1.0000x reference)
"""Bidirectional Mamba block (BiT_MamSleep) on 8 TRN2 NeuronCores — v9.

Sharding: core c handles (batch b = c//2, direction dir = c%2); pairwise
AllReduce joins the two directions; both cores compute the tail redundantly.

s-major scan layout: 32 tiles of [128 part = d (one half of d_inner),
free = t], one per (half h, state s).  dA_s comes straight from ACT exp with
per-partition scale A[:, s]; B/C are row-broadcast per state; the sum over s
is identity-matmul PSUM accumulation on PE.  The depthwise conv is folded
into the in-projection (4 shifted-AP matmuls).  All matmuls bf16.

v9: prechain and scans are chunked over t in two PW=1024 column chunks —
the first scan state starts after the first chunk of the projection chain,
and the out-projection's forward taps run at the half's chunk boundary so
only the reversed taps + eviction sit between scan end and the AllReduce
trigger.  Scan-phase elementwise ops are bf16 tensor_tensor (2x DVE mode);
the scan itself chains across chunks via per-state f32 tail carries.
"""
import sys

if '/opt/trn_rl_repo' not in sys.path:
    sys.path.insert(0, '/opt/trn_rl_repo')

import ml_dtypes
import numpy as np

import concourse.bass as bass
import concourse.bacc as bacc
import concourse.tile as tile
from concourse import mybir
from concourse.bass_utils import run_bass_kernel_spmd

HID = 128
BATCH = 4
SEQ = 2048
D_STATE = 16
D_CONV = 4
D_INNER = 256
DT_RANK = 8

L = SEQ
C = HID
CW = 512
NCH = L // CW
PW = 1024
NPW = L // PW
f32 = mybir.dt.float32
bf16 = mybir.dt.bfloat16
mult = mybir.AluOpType.mult
add = mybir.AluOpType.add
sub = mybir.AluOpType.subtract
AF = mybir.ActivationFunctionType

_PROGRAM = None


def _declare(nc):
    dpf = lambda name, shape: nc.declare_dram_parameter(name, list(shape), f32,
                                                        isOutput=False)
    dph = lambda name, shape: nc.declare_dram_parameter(name, list(shape), bf16,
                                                        isOutput=False)
    p = {}
    p['x'] = dpf('x', (C, L))
    for n in ('wlmT', 'wlgT', 'wcT', 'loT'):
        p[n] = dph(n, (C, C))
    for h in range(2):
        for k in range(D_CONV):
            p[f'wk{h}{k}'] = dph(f'wk{h}{k}', (C, C))
        p[f'inwzT{h}'] = dph(f'inwzT{h}', (C, C))
        p[f'owTA{h}'] = dph(f'owTA{h}', (128, C))
        p[f'owTB{h}'] = dph(f'owTB{h}', (128, C))
        p[f'xpwT{h}'] = dph(f'xpwT{h}', (128, DT_RANK + 2 * D_STATE))
        p[f'dtwT{h}'] = dph(f'dtwT{h}', (DT_RANK, 128))
    p['ident'] = dph('ident', (128, 128))
    for h in range(2):
        p[f'diagD{h}'] = dph(f'diagD{h}', (128, 128))
    p['avec'] = dpf('avec', (128, 32))
    for n in ('conv_b', 'dt_b'):
        p[n] = dpf(n, (128, 2))
    for n in ('bias_lm', 'bias_lg', 'bias_c', 'lo_b', 'ln_g', 'ln_b'):
        p[n] = dpf(n, (C, 1))
    p['y'] = nc.declare_dram_parameter('y', [C, L], f32, isOutput=True)
    return p


class B:
    pass


def _ln_stats_mm(b, x_sb, ones_in, out_bf, lnt, cis):
    """LayerNorm over the 128 channels per column; stage-major to keep the
    ACT table set stable.  Processes only the CW chunks listed in cis."""
    nc = b.nc
    rows_bf, rows_f, nrm0, sq2 = lnt
    ex = rows_bf[0:1, :]
    rr = rows_f[0:1, :]
    csl = [slice(ci * CW, (ci + 1) * CW) for ci in cis]
    for cs in csl:
        ps0 = b.ps.tile([1, CW], f32, name='bank', tag='bank')
        nc.tensor.matmul(ps0, ones_in, x_sb[:, cs], start=True, stop=True)
        nc.scalar.activation(ex[:, cs], ps0, AF.Identity, bias=0.0, scale=1.0 / C)
    for cs in csl:
        psb = b.ps.tile([128, CW], f32, name='bank', tag='bank')
        nc.tensor.matmul(psb, b.ones_row, ex[:, cs], start=True, stop=True)
        nc.vector.scalar_tensor_tensor(nrm0[:, cs], x_sb[:, cs], 1.0, psb,
                                       mult, sub)
    for cs in csl:
        nc.scalar.activation(sq2[:, cs], nrm0[:, cs], AF.Square)
    for cs in csl:
        psv = b.ps.tile([1, CW], f32, name='bank', tag='bank')
        nc.tensor.matmul(psv, b.ones_col, sq2[:, cs], start=True, stop=True)
        nc.scalar.activation(rr[:, cs], psv, AF.Ln, bias=b.eps_t[:, :],
                             scale=1.0 / C)
    for cs in csl:
        nc.scalar.activation(ex[:, cs], rr[:, cs], AF.Exp, bias=0.0, scale=-0.5)
    for cs in csl:
        psr = b.ps.tile([128, CW], f32, name='bank', tag='bank')
        nc.tensor.matmul(psr, b.ones_row, ex[:, cs], start=True, stop=True)
        nc.vector.scalar_tensor_tensor(out_bf[:, cs], nrm0[:, cs], 1.0, psr,
                                       mult, mult)


def _proj(b, lhsT, rhs, out, func, bias, rows=C, out_off=0, pis=None):
    nc = b.nc
    for pi in (range(NPW) if pis is None else pis):
        ps = b.ps.tile([rows, PW], f32, name='bank', tag='bank')
        for half in range(2):
            cs = slice(pi * PW + half * CW, pi * PW + (half + 1) * CW)
            nc.tensor.matmul(ps[:, half * CW:(half + 1) * CW], lhsT, rhs[:, cs],
                             start=True, stop=True)
        ocs = slice(out_off + pi * PW, out_off + (pi + 1) * PW)
        nc.scalar.activation(out[:, ocs], ps, func, bias=bias)


def _build_body(nc, tc, p, ctx):
    b = B()
    b.nc = nc
    b.io = ctx.enter_context(tc.tile_pool(name='io', bufs=1))
    b.pb = ctx.enter_context(tc.tile_pool(name='pb', bufs=1))
    b.pf = ctx.enter_context(tc.tile_pool(name='pf', bufs=2))
    b.bc = ctx.enter_context(tc.tile_pool(name='bc', bufs=6))
    b.cb = ctx.enter_context(tc.tile_pool(name='cb', bufs=6))
    b.da = ctx.enter_context(tc.tile_pool(name='da', bufs=3))
    b.du = ctx.enter_context(tc.tile_pool(name='du', bufs=3))
    b.ht = ctx.enter_context(tc.tile_pool(name='ht', bufs=3))
    b.yc = ctx.enter_context(tc.tile_pool(name='yc', bufs=3))
    b.yq = ctx.enter_context(tc.tile_pool(name='yq', bufs=2))
    b.ya = ctx.enter_context(tc.tile_pool(name='ya', bufs=1))
    b.ps = ctx.enter_context(tc.tile_pool(name='ps', bufs=2, space='PSUM'))
    b.py = ctx.enter_context(tc.tile_pool(name='py', bufs=1, space='PSUM'))
    b.dram = ctx.enter_context(tc.tile_pool(name='drm', bufs=1, space='DRAM'))

    x = b.pf.tile([C, L], f32, name='x', tag='f')
    for ci in range(NCH):
        cs = slice(ci * CW, (ci + 1) * CW)
        nc.sync.dma_start(out=x[:, cs], in_=p['x'][:, cs])

    W = {}
    wspec = [('wlmT', (C, C)), ('wlgT', (C, C)), ('wcT', (C, C)),
             ('loT', (C, C)), ('ident', (128, 128)),
             ('diagD0', (128, 128)), ('diagD1', (128, 128))]
    for h in range(2):
        wspec += [(f'wk{h}{k}', (C, C)) for k in range(D_CONV)]
        wspec += [(f'inwzT{h}', (C, C)), (f'owTA{h}', (128, C)),
                  (f'owTB{h}', (128, C)),
                  (f'xpwT{h}', (128, 40)), (f'dtwT{h}', (8, 128))]
    for n, shape in wspec:
        if n.startswith('dtwT'):
            W[n] = b.io.tile([40, shape[1]], bf16, name=n, tag=n)
            nc.sync.dma_start(out=W[n][32:40, :], in_=p[n][:, :])
            W[n] = W[n][32:40, :]
        else:
            W[n] = b.io.tile(list(shape), bf16, name=n, tag=n)
            nc.sync.dma_start(out=W[n], in_=p[n][:, :])
    V = {}
    V['avec'] = b.io.tile([128, 32], f32, name='avec', tag='avec')
    nc.sync.dma_start(out=V['avec'], in_=p['avec'][:, :])
    for n in ('conv_b', 'dt_b'):
        V[n] = b.io.tile([128, 2], f32, name=n, tag=n)
        nc.sync.dma_start(out=V[n], in_=p[n][:, :])
    for n in ('bias_lm', 'bias_lg', 'bias_c', 'lo_b', 'ln_g', 'ln_b'):
        V[n] = b.io.tile([C, 1], f32, name=n, tag=n)
        nc.sync.dma_start(out=V[n], in_=p[n][:, :])
    ones_col = b.io.tile([C, 1], bf16, name='ones_col', tag='ones_col')
    nc.vector.memset(ones_col, 1.0)
    b.ones_col = ones_col
    ones_colf = b.io.tile([C, 1], f32, name='ones_colf', tag='ones_colf')
    nc.vector.memset(ones_colf, 1.0)
    ones_row = b.io.tile([1, 128], bf16, name='ones_row', tag='ones_row')
    nc.vector.memset(ones_row, 1.0)
    b.ones_row = ones_row
    eps_t = b.io.tile([1, 1], f32, name='lneps', tag='lneps')
    nc.vector.memset(eps_t, 1e-5)
    b.eps_t = eps_t
    tails = [b.io.tile([128, D_STATE], f32, name=f'tails{h}', tag=f'tails{h}')
             for h in range(2)]

    # ---- P1 + P2, chunked over t in PW chunks ----
    nrm = b.pb.tile([C, L], bf16, name='nrm', tag='nrmo')
    lnt_in = (b.pb.tile([2, L], bf16, name='lnb1', tag='lnb'),
              b.pb.tile([1, L], f32, name='lnf1', tag='lnf'),
              b.pb.tile([C, L], bf16, name='nrm01', tag='xmf'),
              b.pb.tile([C, L], bf16, name='sq21', tag='y0'))
    xmf = lnt_in[2]  # lm output overwrites the LN scratch in place
    xm_pad = b.pb.tile([C, D_CONV - 1 + L], bf16, name='xm_pad', tag='xm_pad')
    nc.vector.memset(xm_pad[:, 0:D_CONV - 1], 0.0)
    uc = [b.pb.tile([128, L], bf16, name=f'uc{h}', tag=f'uc{h}')
          for h in range(2)]
    # dbl rows: 0-15 B, 16-31 C, 32-39 dtr (xp_w rows reordered host-side)
    dbl_sb = b.pb.tile([40, L], bf16, name='dbl_sb', tag='dbl_sb')
    dtr = dbl_sb[32:40, :]
    bc_d = b.dram.tile([32, L], bf16, name='bc_d', tag='bc_d')
    dt = [b.pb.tile([128, L], f32, name=f'dt{h}', tag=f'dt{h}')
          for h in range(2)]
    dtu = [b.pb.tile([128, L], bf16, name=f'dtu{h}', tag=f'dtu{h}')
           for h in range(2)]

    for pi in range(NPW):
        pcs = slice(pi * PW, (pi + 1) * PW)
        _ln_stats_mm(b, x, ones_colf, nrm, lnt_in, cis=[2 * pi, 2 * pi + 1])
        _proj(b, W['wlmT'], nrm, xmf, AF.Identity, V['bias_lm'][:, :],
              pis=[pi])
        _proj(b, W['wcT'], xmf, xm_pad, AF.Silu, V['bias_c'][:, :],
              out_off=D_CONV - 1, pis=[pi])
        for h in range(2):
            psu = b.ps.tile([128, PW], f32, name='bank', tag='bank')
            for half in range(2):
                base = pi * PW + half * CW
                for k in range(D_CONV):
                    nc.tensor.matmul(psu[:, half * CW:(half + 1) * CW],
                                     W[f'wk{h}{k}'],
                                     xm_pad[:, k + base:k + base + CW],
                                     start=(k == 0), stop=(k == D_CONV - 1))
            nc.scalar.activation(uc[h][:, pcs], psu, AF.Silu,
                                 bias=V['conv_b'][:, h:h + 1])
        psd = b.ps.tile([40, PW], f32, name='bank', tag='bank')
        for half in range(2):
            hs = slice(half * CW, (half + 1) * CW)
            cs = slice(pi * PW + half * CW, pi * PW + (half + 1) * CW)
            nc.tensor.matmul(psd[:, hs], W['xpwT0'], uc[0][:, cs],
                             start=True, stop=False)
            nc.tensor.matmul(psd[:, hs], W['xpwT1'], uc[1][:, cs],
                             start=False, stop=True)
        nc.scalar.activation(dbl_sb[:, pcs], psd, AF.Identity, bias=0.0)
        nc.sync.dma_start(out=bc_d[:, pcs], in_=dbl_sb[0:32, pcs])
        # dt = ln(1 + exp(dt_w @ dtr + dt_b)); f32 feeds the da exps, bf16
        # feeds the 2x-mode dtu multiply
        for h in range(2):
            z1 = b.pf.tile([128, PW], f32, name=f'z1{h}{pi}', tag='z1')
            _proj(b, W[f'dtwT{h}'], dtr, z1, AF.Exp, V['dt_b'][:, h:h + 1],
                  rows=128, out_off=-pi * PW, pis=[pi])
            nc.scalar.activation(dt[h][:, pcs], z1, AF.Ln, bias=1.0, scale=1.0)
            dtb = b.pf.tile([128, PW], bf16, name=f'dtb{h}{pi}', tag='dtb')
            nc.scalar.activation(dtb, z1, AF.Ln, bias=1.0, scale=1.0)
            nc.vector.tensor_tensor(dtu[h][:, pcs], dtb, uc[h][:, pcs], mult)

    sz = []
    yz = []

    def scan_block(h):
        """Chunk-major chunked scan: all states on chunk 0, then chunk 1
        (chained via f32 tail carries).  psy accumulates per chunk-column
        with the uc*D diagonal matmul closing each bank group."""
        psy = b.py.tile([128, L], f32, name='psy', tag='psy')
        for ci in range(NPW):
            pcs = slice(ci * PW, (ci + 1) * PW)
            for s in range(D_STATE):
                if h == 0 and ci == 0 and s == 8:
                    # gate/z projections dispatched mid-chunk: their ACT
                    # evictions land where the da stream has slack
                    post_h0_kickoff()
                j = 16 * h + s
                b_bc = b.bc.tile([128, PW], bf16, name='b_bc', tag='b_bc')
                src = bass.AP(tensor=bc_d.tensor,
                              offset=bc_d.offset + s * L + ci * PW,
                              ap=[[0, 128], [1, PW]])
                nc.sync.dma_start(out=b_bc, in_=src)
                c_bc = b.cb.tile([128, PW], bf16, name='c_bc', tag='c_bc')
                src = bass.AP(tensor=bc_d.tensor,
                              offset=bc_d.offset + (16 + s) * L + ci * PW,
                              ap=[[0, 128], [1, PW]])
                nc.sync.dma_start(out=c_bc, in_=src)

                da = b.da.tile([128, PW], f32, name='da', tag='da')
                nc.scalar.activation(da, dt[h][:, pcs], AF.Exp, bias=0.0,
                                     scale=V['avec'][:, j:j + 1])
                dbu = b.du.tile([128, PW], bf16, name='dbu', tag='dbu')
                nc.vector.tensor_tensor(dbu, dtu[h][:, pcs], b_bc, mult)
                ht = b.ht.tile([128, PW], bf16, name='ht', tag='ht')
                init = 0.0 if ci == 0 else tails[h][:, s:s + 1]
                nc.vector.tensor_tensor_scan(ht, da, dbu, init, mult, add)
                if ci + 1 < NPW:
                    nc.scalar.activation(tails[h][:, s:s + 1],
                                         ht[:, PW - 1:PW], AF.Identity,
                                         bias=0.0)
                ycm = b.yc.tile([128, PW], bf16, name='ycm', tag='ycm')
                nc.vector.tensor_tensor(ycm, ht, c_bc, mult)
                for half in range(2):
                    cs = slice(half * CW, (half + 1) * CW)
                    nc.tensor.matmul(psy[:, ci * PW + half * CW:
                                         ci * PW + (half + 1) * CW],
                                     W['ident'], ycm[:, cs],
                                     start=(s == 0), stop=False,
                                     skip_group_check=True)
            # fold uc * D into psy on PE (diagonal weights), closing the
            # accumulation group for this chunk's banks
            for half in range(2):
                cs = slice(ci * PW + half * CW, ci * PW + (half + 1) * CW)
                nc.tensor.matmul(psy[:, cs], W[f'diagD{h}'], uc[h][:, cs],
                                 start=False, stop=True,
                                 skip_group_check=True)
            chunk_done(h, psy, ci)
        return psy

    yqs = {}
    ya_parts = {}

    def chunk_done(h, psy, ci):
        """After chunk ci of half h closes: evict that chunk of psy, gate it
        with sz, and run the out-projection taps this chunk feeds: chunk 0
        feeds the forward taps of out-chunk 0 and the reversed taps of
        out-chunk 1 (evicted to SBUF partials, re-injected via an identity
        matmul at the end); chunk 1 completes both out-chunks and triggers
        the pairwise AllReduce."""
        pcs = slice(ci * PW, (ci + 1) * PW)
        yq = b.yq.tile([128, PW], bf16, name='yqc', tag='yqc')
        nc.scalar.activation(yq, psy[:, pcs], AF.Identity, bias=0.0)
        if len(yz) <= h:
            yz.append(b.pb.tile([128, L], bf16, name=f'yz{h}', tag=f'yz{h}'))
        yzt = yz[h]
        nc.vector.tensor_tensor(yzt[:, pcs], yq, sz[h][:, pcs], mult)
        if ci == 0:
            # partial taps from yz chunk 0, evicted to SBUF immediately
            ya = b.ya.tile([C, L], bf16, name=f'ya{h}', tag='ya')
            ya_parts[h] = ya
            for po, wn in ((0, f'owTA{h}'), (1, f'owTB{h}')):
                ps = b.ps.tile([C, PW], f32, name='bank', tag='bank')
                for half in range(2):
                    hs = slice(half * CW, (half + 1) * CW)
                    a0 = half * CW
                    if po == 0:
                        rhs = yzt[:, a0:a0 + CW]
                    else:
                        a1 = PW + half * CW
                        rhs = yzt[:, L - a1 - CW:L - a1][:, ::-1]
                    nc.tensor.matmul(ps[:, hs], W[wn], rhs,
                                     start=True, stop=True)
                nc.scalar.activation(ya[:, po * PW:(po + 1) * PW], ps,
                                     AF.Identity, bias=0.0)
        else:
            ya = ya_parts[h]
            y_out = b.pb.tile([C, L], bf16, name=f'y{h}t', tag=f'y{h}t')
            cc_in = b.dram.tile([C, L], bf16, name=f'cc_in{h}', tag=f'cc_in{h}')
            for po in range(NPW):
                ps = b.ps.tile([C, PW], f32, name='bank', tag='bank')
                for half in range(2):
                    hs = slice(half * CW, (half + 1) * CW)
                    nc.tensor.matmul(
                        ps[:, hs], W['ident'],
                        ya[:, po * PW + half * CW:po * PW + (half + 1) * CW],
                        start=True, stop=False)
                for half in range(2):
                    hs = slice(half * CW, (half + 1) * CW)
                    a0 = po * PW + half * CW
                    if po == 0:
                        rcs = yzt[:, L - a0 - CW:L - a0][:, ::-1]
                        nc.tensor.matmul(ps[:, hs], W[f'owTB{h}'], rcs,
                                         start=False, stop=True)
                    else:
                        nc.tensor.matmul(ps[:, hs], W[f'owTA{h}'],
                                         yzt[:, a0:a0 + CW], start=False,
                                         stop=True)
                ocs = slice(po * PW, (po + 1) * PW)
                nc.scalar.activation(y_out[:, ocs], ps, AF.Identity,
                                     bias=0.0)
                nc.sync.dma_start(out=cc_in[:, ocs], in_=y_out[:, ocs])
            cc_out = b.dram.tile([C, L], bf16, name=f'cc_out{h}',
                                 tag=f'cc_out{h}')
            nc.gpsimd.collective_compute(
                'AllReduce', add,
                replica_groups=[[0, 1], [2, 3], [4, 5], [6, 7]],
                ins=[cc_in.opt()], outs=[cc_out.opt()])
            cc_outs.append(cc_out)

    cc_outs = []

    def post_h0_kickoff():
        """gate/z projections dispatched mid h0-scan — their PE/ACT work
        hides under the remaining scan chunks."""
        for h in range(2):
            szt = b.pb.tile([128, L], bf16, name=f'sz{h}', tag=f'sz{h}')
            _proj(b, W[f'inwzT{h}'],
                  xm_pad[:, D_CONV - 1:D_CONV - 1 + L], szt, AF.Silu, 0.0)
            sz.append(szt)
        gate = b.pb.tile([C, L], bf16, name='gate', tag='gate')
        _proj(b, W['wlgT'], nrm, gate, AF.Silu, V['bias_lg'][:, :])
        b.gate = gate

    scan_block(0)
    scan_block(1)
    gate = b.gate

    # ---- P5, split so the h0 part runs in AR1's latency shadow ----
    y_sum = b.pb.tile([C, L], bf16, name='y_sum', tag='y0')
    nc.sync.dma_start(out=y_sum, in_=cc_outs[0])
    g1a = b.pb.tile([C, L], bf16, name='g1a', tag='nrmo')
    nc.vector.tensor_tensor(g1a, y_sum, gate, mult)
    ps_t2 = []
    for pi in range(NPW):
        ps = b.ps.tile([C, PW], f32, name='bank', tag='bank')
        for half in range(2):
            cs = slice(pi * PW + half * CW, pi * PW + (half + 1) * CW)
            nc.tensor.matmul(ps[:, half * CW:(half + 1) * CW], W['loT'],
                             g1a[:, cs], start=True, stop=False,
                             skip_group_check=True)
        ps_t2.append(ps)
    ysum2 = b.pb.tile([C, L], bf16, name='ysum2', tag='ysum2')
    nc.sync.dma_start(out=ysum2, in_=cc_outs[1])
    g1b = b.pb.tile([C, L], bf16, name='g1b', tag='y1t')
    nc.vector.tensor_tensor(g1b, ysum2, gate, mult)
    t2 = b.pb.tile([C, L], bf16, name='t2', tag='t2')
    for pi in range(NPW):
        for half in range(2):
            cs = slice(pi * PW + half * CW, pi * PW + (half + 1) * CW)
            nc.tensor.matmul(ps_t2[pi][:, half * CW:(half + 1) * CW], W['loT'],
                             g1b[:, cs], start=False, stop=True,
                             skip_group_check=True)
        ocs = slice(pi * PW, (pi + 1) * PW)
        nc.scalar.activation(t2[:, ocs], ps_t2[pi], AF.Identity,
                             bias=V['lo_b'][:, :])

    o1 = b.pb.tile([C, L], bf16, name='o1', tag='xm_pad')
    lnt_out = (lnt_in[0], lnt_in[1],
               b.pb.tile([C, L], bf16, name='nrm02', tag='xmf'),
               b.pb.tile([C, L], bf16, name='sq22', tag='y0'))
    _ln_stats_mm(b, t2, b.ones_col, o1, lnt_out, cis=list(range(NCH)))
    out_sb = b.pf.tile([C, L], f32, name='out_sb', tag='f')
    nc.vector.tensor_scalar(out_sb, o1, V['ln_g'][:, :], V['ln_b'][:, :],
                            mult, add)
    nc.sync.dma_start(out=p['y'][:, :], in_=out_sb)


def _build_program():
    import contextlib
    nc = bacc.Bacc('TRN2', target_bir_lowering=False, debug=False, num_devices=8)
    p = _declare(nc)
    with tile.TileContext(nc) as tc:
        with contextlib.ExitStack() as ctx:
            _build_body(nc, tc, p, ctx)
    nc.compile()
    return nc


def _prep_core_inputs(inputs, bidx, d):
    g = lambda n: np.asarray(inputs[n], dtype=np.float32)
    x = g('x')
    ln_g = g('ln_g')
    ln_b = g('ln_b')
    pre = 'mf_' if d == 0 else 'mb_'
    P = lambda n: np.asarray(inputs[pre + n], dtype=np.float32)

    lm_w, lm_b = g('lm_w'), g('lm_b')
    lg_w, lg_b = g('lg_w'), g('lg_b')
    lo_w, lo_b = g('lo_w'), g('lo_b')
    if d == 0:
        wc, cb = g('cf_w'), g('cf_b')
    else:
        wc, cb = np.ascontiguousarray(g('cb_w')[:, ::-1]), g('cb_b')

    A = -np.exp(P('Alog'))
    avec = np.zeros((128, 32), np.float32)
    for h in range(2):
        for s in range(16):
            avec[:, 16 * h + s] = A[128 * h:128 * (h + 1), s]

    bf = lambda a: np.ascontiguousarray(np.asarray(a, dtype=ml_dtypes.bfloat16))
    col = lambda v: np.ascontiguousarray(v.astype(np.float32).reshape(-1, 1))
    halves = lambda v: np.ascontiguousarray(
        np.stack([v[:128], v[128:]], axis=1).astype(np.float32))
    T = lambda w: np.ascontiguousarray(w.T)

    in_w = P('in_w')
    conv_w = P('conv_w')
    xpw = P('xp_w')
    xpw = np.concatenate([xpw[DT_RANK:], xpw[:DT_RANK]], axis=0)
    xpwT = np.ascontiguousarray(xpw.T)
    outwT = np.ascontiguousarray(P('out_w').T)
    dtwT = np.ascontiguousarray(P('dt_w').T)

    out = {
        'x': np.ascontiguousarray(x[bidx]),
        'wlmT': bf(T(lm_w * ln_g[None, :])),
        'wlgT': bf(T(lg_w * ln_g[None, :])),
        'wcT': bf(T(wc)),
        'loT': bf(T(lo_w)),
        'ident': bf(np.eye(128, dtype=np.float32)),
        'avec': avec,
        'conv_b': halves(P('conv_b')),
        'dt_b': halves(P('dt_b')),
        'bias_lm': col(lm_w @ ln_b + lm_b),
        'bias_lg': col(lg_w @ ln_b + lg_b),
        'bias_c': col(cb),
        'lo_b': col(lo_b),
        'ln_g': col(ln_g),
        'ln_b': col(ln_b),
    }
    for h in range(2):
        hsl = slice(128 * h, 128 * (h + 1))
        out[f'diagD{h}'] = bf(np.diag(P('D')[hsl]).astype(np.float32))
        for k in range(D_CONV):
            wk = in_w[hsl, :] * conv_w[hsl, k:k + 1]
            out[f'wk{h}{k}'] = bf(T(wk))
        out[f'inwzT{h}'] = bf(T(P('in_w')[256:][hsl, :]))
        ow = outwT[hsl, :]
        out[f'owTA{h}'] = bf(ow if d == 0 else np.zeros_like(ow))
        out[f'owTB{h}'] = bf(np.zeros_like(ow) if d == 0 else ow)
        out[f'xpwT{h}'] = bf(xpwT[hsl, :])
        out[f'dtwT{h}'] = bf(dtwT[:, hsl])
    return out


def get_program():
    global _PROGRAM
    if _PROGRAM is None:
        _PROGRAM = _build_program()
    return _PROGRAM


def run(inputs, **run_kwargs):
    nc = get_program()
    in_maps = [_prep_core_inputs(inputs, c // 2, c % 2) for c in range(8)]
    res = run_bass_kernel_spmd(nc, in_maps, core_ids=list(range(8)), **run_kwargs)
    out = np.stack([res.results[2 * b]['y'] for b in range(BATCH)], axis=0)
    return out, res


def kernel(**inputs) -> np.ndarray:
    out, _ = run(inputs)
    return out.astype(np.float32)


# revision 31
# speedup vs baseline: 1.0907x; 1.0907x over previous
"""Bidirectional Mamba block (BiT_MamSleep) on 8 TRN2 NeuronCores — v10.

Sharding: core c handles (batch b = c//2, direction dir = c%2); pairwise
AllReduce joins the two directions; both cores compute the tail redundantly.

s-major scan layout: 32 tiles of [128 part = d (one half of d_inner),
free = t], one per (half h, state s).  dA_s comes straight from ACT exp with
per-partition scale A[:, s]; B/C are row-broadcast per state; the sum over s
is identity-matmul PSUM accumulation on PE.  The depthwise conv is folded
into the in-projection (4 shifted-AP matmuls).  All matmuls bf16.

Scan-phase elementwise ops are bf16 tensor_tensor (the only DVE op family
with a 2x perf-mode uop; scalar_tensor_tensor and the scan itself run 1x).
uc*D rides the psy PSUM accumulation as a diagonal matmul.  The h0
AllReduce hides under the h1 scan; the lo-projection of the h0 term runs
inside the h1 AllReduce's ~35 us latency shadow.
"""
import sys

if '/opt/trn_rl_repo' not in sys.path:
    sys.path.insert(0, '/opt/trn_rl_repo')

import ml_dtypes
import numpy as np

import concourse.bass as bass
import concourse.bacc as bacc
import concourse.tile as tile
from concourse import mybir
from concourse.bass_utils import run_bass_kernel_spmd

HID = 128
BATCH = 4
SEQ = 2048
D_STATE = 16
D_CONV = 4
D_INNER = 256
DT_RANK = 8

L = SEQ
C = HID
CW = 512
NCH = L // CW
PW = 1024
NPW = L // PW
f32 = mybir.dt.float32
bf16 = mybir.dt.bfloat16
mult = mybir.AluOpType.mult
add = mybir.AluOpType.add
sub = mybir.AluOpType.subtract
AF = mybir.ActivationFunctionType

_PROGRAM = None


def _declare(nc):
    dpf = lambda name, shape: nc.declare_dram_parameter(name, list(shape), f32,
                                                        isOutput=False)
    dph = lambda name, shape: nc.declare_dram_parameter(name, list(shape), bf16,
                                                        isOutput=False)
    p = {}
    p['x'] = dpf('x', (C, L))
    for n in ('wlmT', 'wlgT', 'wcT', 'loT'):
        p[n] = dph(n, (C, C))
    for h in range(2):
        for k in range(D_CONV):
            p[f'wk{h}{k}'] = dph(f'wk{h}{k}', (C, C))
        p[f'inwzT{h}'] = dph(f'inwzT{h}', (C, C))
        p[f'owTA{h}'] = dph(f'owTA{h}', (128, C))
        p[f'owTB{h}'] = dph(f'owTB{h}', (128, C))
        p[f'xpwT{h}'] = dph(f'xpwT{h}', (128, DT_RANK + 2 * D_STATE))
        p[f'dtwT{h}'] = dph(f'dtwT{h}', (DT_RANK, 128))
    p['ident'] = dph('ident', (128, 128))
    for h in range(2):
        p[f'diagD{h}'] = dph(f'diagD{h}', (128, 128))
    p['avec'] = dpf('avec', (128, 32))
    for n in ('conv_b', 'dt_b'):
        p[n] = dpf(n, (128, 2))
    for n in ('bias_lm', 'bias_lg', 'bias_c', 'lo_b', 'ln_g', 'ln_b'):
        p[n] = dpf(n, (C, 1))
    p['y'] = nc.declare_dram_parameter('y', [C, L], f32, isOutput=True)
    return p


class B:
    pass


def _ln_stats_mm(b, x_mm, x_sub, out_bf):
    """LayerNorm over the 128 channels per column; stage-major to keep the
    ACT table set stable.  x_mm must be bf16 (fast-path PE matmuls); x_sub
    feeds the mean-subtract and may be f32 for precision."""
    nc = b.nc
    rows_bf = b.pb.tile([2, L], bf16, name='lnrowsb', tag='lnb')
    rows_f = b.pb.tile([1, L], f32, name='lnrowsf', tag='lnf')
    ex = rows_bf[0:1, :]
    rr = rows_f[0:1, :]
    nrm0 = b.pb.tile([C, L], bf16, name='nrm0', tag='xmf')
    sq2 = b.pb.tile([C, L], bf16, name='sq2', tag='y0')
    csl = [slice(ci * CW, (ci + 1) * CW) for ci in range(NCH)]
    for cs in csl:
        ps0 = b.ps.tile([1, CW], f32, name='bank', tag='bank')
        nc.tensor.matmul(ps0, b.ones_col, x_mm[:, cs], start=True, stop=True)
        nc.scalar.activation(ex[:, cs], ps0, AF.Identity, bias=0.0, scale=1.0 / C)
    for cs in csl:
        psb = b.ps.tile([128, CW], f32, name='bank', tag='bank')
        nc.tensor.matmul(psb, b.ones_row, ex[:, cs], start=True, stop=True)
        nc.vector.scalar_tensor_tensor(nrm0[:, cs], x_sub[:, cs], 1.0, psb,
                                       mult, sub)
    for cs in csl:
        nc.scalar.activation(sq2[:, cs], nrm0[:, cs], AF.Square)
    for cs in csl:
        psv = b.ps.tile([1, CW], f32, name='bank', tag='bank')
        nc.tensor.matmul(psv, b.ones_col, sq2[:, cs], start=True, stop=True)
        nc.scalar.activation(rr[:, cs], psv, AF.Ln, bias=b.eps_t[:, :],
                             scale=1.0 / C)
    for cs in csl:
        nc.scalar.activation(ex[:, cs], rr[:, cs], AF.Exp, bias=0.0, scale=-0.5)
    for cs in csl:
        psr = b.ps.tile([128, CW], f32, name='bank', tag='bank')
        nc.tensor.matmul(psr, b.ones_row, ex[:, cs], start=True, stop=True)
        nc.vector.scalar_tensor_tensor(out_bf[:, cs], nrm0[:, cs], 1.0, psr,
                                       mult, mult)


def _proj(b, lhsT, rhs, out, func, bias, rows=C, out_off=0):
    nc = b.nc
    for pi in range(NPW):
        ps = b.ps.tile([rows, PW], f32, name='bank', tag='bank')
        for half in range(2):
            cs = slice(pi * PW + half * CW, pi * PW + (half + 1) * CW)
            nc.tensor.matmul(ps[:, half * CW:(half + 1) * CW], lhsT, rhs[:, cs],
                             start=True, stop=True)
        ocs = slice(out_off + pi * PW, out_off + (pi + 1) * PW)
        nc.scalar.activation(out[:, ocs], ps, func, bias=bias)


def _build_body(nc, tc, p, ctx):
    b = B()
    b.nc = nc
    b.io = ctx.enter_context(tc.tile_pool(name='io', bufs=1))
    b.pb = ctx.enter_context(tc.tile_pool(name='pb', bufs=1))
    b.pf = ctx.enter_context(tc.tile_pool(name='pf', bufs=2))
    b.bc = ctx.enter_context(tc.tile_pool(name='bc', bufs=3))
    b.cb = ctx.enter_context(tc.tile_pool(name='cb', bufs=3))
    b.da = ctx.enter_context(tc.tile_pool(name='da', bufs=2))
    b.du = ctx.enter_context(tc.tile_pool(name='du', bufs=3))
    b.ht = ctx.enter_context(tc.tile_pool(name='ht', bufs=3))
    b.yc = ctx.enter_context(tc.tile_pool(name='yc', bufs=3))
    b.ps = ctx.enter_context(tc.tile_pool(name='ps', bufs=2, space='PSUM'))
    b.py = ctx.enter_context(tc.tile_pool(name='py', bufs=1, space='PSUM'))
    b.dram = ctx.enter_context(tc.tile_pool(name='drm', bufs=1, space='DRAM'))

    # LN-critical state first: the x DMA, the ones/eps memsets and the
    # input layernorm only need these — the ~30-deep weight DMA issue
    # stream on the sync queue then runs behind the LN instead of in
    # front of it.
    x = b.pf.tile([C, L], f32, name='x', tag='f')
    for ci in range(NCH):
        cs = slice(ci * CW, (ci + 1) * CW)
        nc.sync.dma_start(out=x[:, cs], in_=p['x'][:, cs])
    ones_col = b.io.tile([C, 1], bf16, name='ones_col', tag='ones_col')
    nc.vector.memset(ones_col, 1.0)
    b.ones_col = ones_col
    ones_row = b.io.tile([1, 128], bf16, name='ones_row', tag='ones_row')
    nc.vector.memset(ones_row, 1.0)
    b.ones_row = ones_row
    eps_t = b.io.tile([1, 1], f32, name='lneps', tag='lneps')
    nc.vector.memset(eps_t, 1e-5)
    b.eps_t = eps_t
    # bf16 copy of x for the fast-path (bf16 weights) LN matmuls
    xb = b.pb.tile([C, L], bf16, name='xb', tag='t2')
    for ci in range(NCH):
        cs = slice(ci * CW, (ci + 1) * CW)
        nc.scalar.activation(xb[:, cs], x[:, cs], AF.Identity, bias=0.0)

    # ---- P1: input layernorm (stats from bf16 x; subtract still f32 x) ----
    nrm = b.pb.tile([C, L], bf16, name='nrm', tag='nrmo')
    _ln_stats_mm(b, xb, x, nrm)

    W = {}
    wspec = [('wlmT', (C, C)), ('wlgT', (C, C)), ('wcT', (C, C)),
             ('loT', (C, C)), ('ident', (128, 128)),
             ('diagD0', (128, 128)), ('diagD1', (128, 128))]
    for h in range(2):
        wspec += [(f'wk{h}{k}', (C, C)) for k in range(D_CONV)]
        wspec += [(f'inwzT{h}', (C, C)), (f'owTA{h}', (128, C)),
                  (f'owTB{h}', (128, C)),
                  (f'xpwT{h}', (128, 40)), (f'dtwT{h}', (8, 128))]
    for n, shape in wspec:
        if n.startswith('dtwT'):
            W[n] = b.io.tile([40, shape[1]], bf16, name=n, tag=n)
            nc.sync.dma_start(out=W[n][32:40, :], in_=p[n][:, :])
            W[n] = W[n][32:40, :]
        else:
            W[n] = b.io.tile(list(shape), bf16, name=n, tag=n)
            nc.sync.dma_start(out=W[n], in_=p[n][:, :])
    V = {}
    V['avec'] = b.io.tile([128, 32], f32, name='avec', tag='avec')
    nc.sync.dma_start(out=V['avec'], in_=p['avec'][:, :])
    for n in ('conv_b', 'dt_b'):
        V[n] = b.io.tile([128, 2], f32, name=n, tag=n)
        nc.sync.dma_start(out=V[n], in_=p[n][:, :])
    for n in ('bias_lm', 'bias_lg', 'bias_c', 'lo_b', 'ln_g', 'ln_b'):
        V[n] = b.io.tile([C, 1], f32, name=n, tag=n)
        nc.sync.dma_start(out=V[n], in_=p[n][:, :])

    # ---- P2 ----
    xmf = b.pb.tile([C, L], bf16, name='xmf', tag='xmf')
    _proj(b, W['wlmT'], nrm, xmf, AF.Identity, V['bias_lm'][:, :])
    xm_pad = b.pb.tile([C, D_CONV - 1 + L], bf16, name='xm_pad', tag='xm_pad')
    nc.vector.memset(xm_pad[:, 0:D_CONV - 1], 0.0)
    _proj(b, W['wcT'], xmf, xm_pad, AF.Silu, V['bias_c'][:, :],
          out_off=D_CONV - 1)

    uc = []
    for h in range(2):
        uct = b.pb.tile([128, L], bf16, name=f'uc{h}', tag=f'uc{h}')
        for pi in range(NPW):
            psu = b.ps.tile([128, PW], f32, name='bank', tag='bank')
            for half in range(2):
                base = pi * PW + half * CW
                for k in range(D_CONV):
                    nc.tensor.matmul(psu[:, half * CW:(half + 1) * CW],
                                     W[f'wk{h}{k}'],
                                     xm_pad[:, k + base:k + base + CW],
                                     start=(k == 0), stop=(k == D_CONV - 1))
            nc.scalar.activation(uct[:, pi * PW:(pi + 1) * PW], psu, AF.Silu,
                                 bias=V['conv_b'][:, h:h + 1])
        uc.append(uct)

    # dbl rows: 0-15 B, 16-31 C, 32-39 dtr (xp_w rows reordered host-side)
    dbl_sb = b.pb.tile([40, L], bf16, name='dbl_sb', tag='dbl_sb')
    dtr = dbl_sb[32:40, :]
    for pi in range(NPW):
        psd = b.ps.tile([40, PW], f32, name='bank', tag='bank')
        for half in range(2):
            hs = slice(half * CW, (half + 1) * CW)
            cs = slice(pi * PW + half * CW, pi * PW + (half + 1) * CW)
            nc.tensor.matmul(psd[:, hs], W['xpwT0'], uc[0][:, cs],
                             start=True, stop=False)
            nc.tensor.matmul(psd[:, hs], W['xpwT1'], uc[1][:, cs],
                             start=False, stop=True)
        nc.scalar.activation(dbl_sb[:, pi * PW:(pi + 1) * PW], psd,
                             AF.Identity, bias=0.0)
    bc_d = b.dram.tile([32, L], bf16, name='bc_d', tag='bc_d')
    nc.sync.dma_start(out=bc_d, in_=dbl_sb[0:32, :])

    # dt = ln(1 + exp(dt_w @ dtr + dt_b)); f32 copy feeds the da exps,
    # bf16 copy feeds the 2x-mode dtu multiply
    dt = []
    dtu = []
    for h in range(2):
        z1 = b.pf.tile([128, L], f32, name=f'z1{h}', tag='f')
        _proj(b, W[f'dtwT{h}'], dtr, z1, AF.Exp, V['dt_b'][:, h:h + 1],
              rows=128)
        dtt = b.pb.tile([128, L], f32, name=f'dt{h}', tag=f'dt{h}')
        nc.scalar.activation(dtt, z1, AF.Ln, bias=1.0, scale=1.0)
        dtt_bf = b.pb.tile([128, L], bf16, name=f'dtb{h}', tag=f'dtb{h}')
        nc.scalar.activation(dtt_bf, z1, AF.Ln, bias=1.0, scale=1.0)
        dt.append(dtt)
        dtut = b.pb.tile([128, L], bf16, name=f'dtu{h}', tag=f'dtu{h}')
        nc.vector.tensor_tensor(dtut, dtt_bf, uc[h], mult)
        dtu.append(dtut)

    sz = []
    yz = []

    def scan_block(h):
        psy = b.py.tile([128, L], f32, name='psy', tag='psy')
        for s in range(D_STATE):
            j = 16 * h + s
            b_bc = b.bc.tile([128, L], bf16, name='b_bc', tag='b_bc')
            src = bass.AP(tensor=bc_d.tensor, offset=bc_d.offset + s * L,
                          ap=[[0, 128], [1, L]])
            nc.sync.dma_start(out=b_bc, in_=src)
            c_bc = b.cb.tile([128, L], bf16, name='c_bc', tag='c_bc')
            src = bass.AP(tensor=bc_d.tensor, offset=bc_d.offset + (16 + s) * L,
                          ap=[[0, 128], [1, L]])
            nc.sync.dma_start(out=c_bc, in_=src)

            da = b.da.tile([128, L], f32, name='da', tag='da')
            nc.scalar.activation(da, dt[h], AF.Exp, bias=0.0,
                                 scale=V['avec'][:, j:j + 1])
            dbu = b.du.tile([128, L], bf16, name='dbu', tag='dbu')
            nc.vector.tensor_tensor(dbu, dtu[h], b_bc, mult)
            ht = b.ht.tile([128, L], bf16, name='ht', tag='ht')
            nc.vector.tensor_tensor_scan(ht, da, dbu, 0.0, mult, add)
            ycm = b.yc.tile([128, L], bf16, name='ycm', tag='ycm')
            nc.vector.tensor_tensor(ycm, ht, c_bc, mult)
            for ci in range(NCH):
                cs = slice(ci * CW, (ci + 1) * CW)
                nc.tensor.matmul(psy[:, cs], W['ident'], ycm[:, cs],
                                 start=(s == 0), stop=False,
                                 skip_group_check=True)
        # fold uc * D into psy on PE (diagonal weights), closing the group
        for ci in range(NCH):
            cs = slice(ci * CW, (ci + 1) * CW)
            nc.tensor.matmul(psy[:, cs], W[f'diagD{h}'], uc[h][:, cs],
                             start=False, stop=True, skip_group_check=True)
        return psy

    def finish_half(h, psy):
        """psy -> yq -> yz -> out-projection -> staging -> AllReduce, all
        at PW granularity so the ACT/DVE/PE/DMA stages pipeline."""
        yq = b.pb.tile([128, L], bf16, name=f'yq{h}', tag=f'dtu{h}')
        yzt = b.pb.tile([128, L], bf16, name=f'yz{h}', tag=f'yz{h}')
        yz.append(yzt)
        for pi in range(NPW):
            pcs = slice(pi * PW, (pi + 1) * PW)
            nc.scalar.activation(yq[:, pcs], psy[:, pcs], AF.Identity,
                                 bias=0.0)
            nc.vector.tensor_tensor(yzt[:, pcs], yq[:, pcs], sz[h][:, pcs],
                                    mult)
        y_out = b.pb.tile([C, L], bf16, name=f'y{h}t', tag=f'y{h}t')
        cc_in = b.dram.tile([C, L], bf16, name=f'cc_in{h}', tag=f'cc_in{h}')
        for pi in range(NPW):
            ps = b.ps.tile([C, PW], f32, name='bank', tag='bank')
            for half in range(2):
                hs = slice(half * CW, (half + 1) * CW)
                a0 = pi * PW + half * CW
                a1 = pi * PW + (half + 1) * CW
                nc.tensor.matmul(ps[:, hs], W[f'owTA{h}'], yzt[:, a0:a1],
                                 start=True, stop=False)
                rcs = yzt[:, L - a1:L - a0][:, ::-1]
                nc.tensor.matmul(ps[:, hs], W[f'owTB{h}'], rcs,
                                 start=False, stop=True)
            ocs = slice(pi * PW, (pi + 1) * PW)
            nc.scalar.activation(y_out[:, ocs], ps, AF.Identity, bias=0.0)
            nc.sync.dma_start(out=cc_in[:, ocs], in_=y_out[:, ocs])
        cc_out = b.dram.tile([C, L], bf16, name=f'cc_out{h}', tag=f'cc_out{h}')
        nc.gpsimd.collective_compute(
            'AllReduce', add,
            replica_groups=[[0, 1], [2, 3], [4, 5], [6, 7]],
            ins=[cc_in.opt()], outs=[cc_out.opt()])
        return cc_out

    # h = 0 scans; gate/z projections run on ACT/PE meanwhile
    psy0 = scan_block(0)
    gate = b.pb.tile([C, L], bf16, name='gate', tag='gate')
    _proj(b, W['wlgT'], nrm, gate, AF.Silu, V['bias_lg'][:, :])
    for h in range(2):
        szt = b.pb.tile([128, L], bf16, name=f'sz{h}', tag=f'sz{h}')
        _proj(b, W[f'inwzT{h}'],
              xm_pad[:, D_CONV - 1:D_CONV - 1 + L], szt, AF.Silu, 0.0)
        sz.append(szt)

    # h=0 out-projection + its AllReduce, hidden under the h=1 scan block
    cc_out0 = finish_half(0, psy0)
    psy1 = scan_block(1)
    cc_out1 = finish_half(1, psy1)

    # ---- P5, split so the h0 part runs in AR1's latency shadow ----
    y_sum = b.pb.tile([C, L], bf16, name='y_sum', tag='y0')
    nc.sync.dma_start(out=y_sum, in_=cc_out0)
    g1a = b.pb.tile([C, L], bf16, name='g1a', tag='nrmo')
    nc.vector.tensor_tensor(g1a, y_sum, gate, mult)
    ps_t2 = []
    for pi in range(NPW):
        ps = b.ps.tile([C, PW], f32, name='bank', tag='bank')
        for half in range(2):
            cs = slice(pi * PW + half * CW, pi * PW + (half + 1) * CW)
            nc.tensor.matmul(ps[:, half * CW:(half + 1) * CW], W['loT'],
                             g1a[:, cs], start=True, stop=False,
                             skip_group_check=True)
        ps_t2.append(ps)
    ysum2 = b.pb.tile([C, L], bf16, name='ysum2', tag='ysum2')
    nc.sync.dma_start(out=ysum2, in_=cc_out1)
    g1b = b.pb.tile([C, L], bf16, name='g1b', tag='y1t')
    nc.vector.tensor_tensor(g1b, ysum2, gate, mult)
    t2 = b.pb.tile([C, L], bf16, name='t2', tag='t2')
    for pi in range(NPW):
        for half in range(2):
            cs = slice(pi * PW + half * CW, pi * PW + (half + 1) * CW)
            nc.tensor.matmul(ps_t2[pi][:, half * CW:(half + 1) * CW], W['loT'],
                             g1b[:, cs], start=False, stop=True,
                             skip_group_check=True)
        ocs = slice(pi * PW, (pi + 1) * PW)
        nc.scalar.activation(t2[:, ocs], ps_t2[pi], AF.Identity,
                             bias=V['lo_b'][:, :])

    o1 = b.pb.tile([C, L], bf16, name='o1', tag='xm_pad')
    _ln_stats_mm(b, t2, t2, o1)
    out_sb = b.pf.tile([C, L], f32, name='out_sb', tag='f')
    nc.vector.tensor_scalar(out_sb, o1, V['ln_g'][:, :], V['ln_b'][:, :],
                            mult, add)
    nc.sync.dma_start(out=p['y'][:, :], in_=out_sb)


def _build_program():
    import contextlib
    nc = bacc.Bacc('TRN2', target_bir_lowering=False, debug=False, num_devices=8)
    p = _declare(nc)
    with tile.TileContext(nc) as tc:
        with contextlib.ExitStack() as ctx:
            _build_body(nc, tc, p, ctx)
    nc.compile()
    return nc


def _prep_core_inputs(inputs, bidx, d):
    g = lambda n: np.asarray(inputs[n], dtype=np.float32)
    x = g('x')
    ln_g = g('ln_g')
    ln_b = g('ln_b')
    pre = 'mf_' if d == 0 else 'mb_'
    P = lambda n: np.asarray(inputs[pre + n], dtype=np.float32)

    lm_w, lm_b = g('lm_w'), g('lm_b')
    lg_w, lg_b = g('lg_w'), g('lg_b')
    lo_w, lo_b = g('lo_w'), g('lo_b')
    if d == 0:
        wc, cb = g('cf_w'), g('cf_b')
    else:
        wc, cb = np.ascontiguousarray(g('cb_w')[:, ::-1]), g('cb_b')

    A = -np.exp(P('Alog'))
    avec = np.zeros((128, 32), np.float32)
    for h in range(2):
        for s in range(16):
            avec[:, 16 * h + s] = A[128 * h:128 * (h + 1), s]

    bf = lambda a: np.ascontiguousarray(np.asarray(a, dtype=ml_dtypes.bfloat16))
    col = lambda v: np.ascontiguousarray(v.astype(np.float32).reshape(-1, 1))
    halves = lambda v: np.ascontiguousarray(
        np.stack([v[:128], v[128:]], axis=1).astype(np.float32))
    T = lambda w: np.ascontiguousarray(w.T)

    in_w = P('in_w')
    conv_w = P('conv_w')
    xpw = P('xp_w')
    xpw = np.concatenate([xpw[DT_RANK:], xpw[:DT_RANK]], axis=0)
    xpwT = np.ascontiguousarray(xpw.T)
    outwT = np.ascontiguousarray(P('out_w').T)
    dtwT = np.ascontiguousarray(P('dt_w').T)

    out = {
        'x': np.ascontiguousarray(x[bidx]),
        'wlmT': bf(T(lm_w * ln_g[None, :])),
        'wlgT': bf(T(lg_w * ln_g[None, :])),
        'wcT': bf(T(wc)),
        'loT': bf(T(lo_w)),
        'ident': bf(np.eye(128, dtype=np.float32)),
        'avec': avec,
        'conv_b': halves(P('conv_b')),
        'dt_b': halves(P('dt_b')),
        'bias_lm': col(lm_w @ ln_b + lm_b),
        'bias_lg': col(lg_w @ ln_b + lg_b),
        'bias_c': col(cb),
        'lo_b': col(lo_b),
        'ln_g': col(ln_g),
        'ln_b': col(ln_b),
    }
    for h in range(2):
        hsl = slice(128 * h, 128 * (h + 1))
        out[f'diagD{h}'] = bf(np.diag(P('D')[hsl]).astype(np.float32))
        for k in range(D_CONV):
            wk = in_w[hsl, :] * conv_w[hsl, k:k + 1]
            out[f'wk{h}{k}'] = bf(T(wk))
        out[f'inwzT{h}'] = bf(T(P('in_w')[256:][hsl, :]))
        ow = outwT[hsl, :]
        out[f'owTA{h}'] = bf(ow if d == 0 else np.zeros_like(ow))
        out[f'owTB{h}'] = bf(np.zeros_like(ow) if d == 0 else ow)
        out[f'xpwT{h}'] = bf(xpwT[hsl, :])
        out[f'dtwT{h}'] = bf(dtwT[:, hsl])
    return out


def get_program():
    global _PROGRAM
    if _PROGRAM is None:
        _PROGRAM = _build_program()
    return _PROGRAM


def run(inputs, **run_kwargs):
    nc = get_program()
    in_maps = [_prep_core_inputs(inputs, c // 2, c % 2) for c in range(8)]
    res = run_bass_kernel_spmd(nc, in_maps, core_ids=list(range(8)), **run_kwargs)
    out = np.stack([res.results[2 * b]['y'] for b in range(BATCH)], axis=0)
    return out, res


def kernel(**inputs) -> np.ndarray:
    out, _ = run(inputs)
    return out.astype(np.float32)


# revision 32
# speedup vs baseline: 1.0932x; 1.0022x over previous
"""Bidirectional Mamba block (BiT_MamSleep) on 8 TRN2 NeuronCores — v10.

Sharding: core c handles (batch b = c//2, direction dir = c%2); pairwise
AllReduce joins the two directions; both cores compute the tail redundantly.

s-major scan layout: 32 tiles of [128 part = d (one half of d_inner),
free = t], one per (half h, state s).  dA_s comes straight from ACT exp with
per-partition scale A[:, s]; B/C are row-broadcast per state; the sum over s
is identity-matmul PSUM accumulation on PE.  The depthwise conv is folded
into the in-projection (4 shifted-AP matmuls).  All matmuls bf16.

Scan-phase elementwise ops are bf16 tensor_tensor (the only DVE op family
with a 2x perf-mode uop; scalar_tensor_tensor and the scan itself run 1x).
uc*D rides the psy PSUM accumulation as a diagonal matmul.  The h0
AllReduce hides under the h1 scan; the lo-projection of the h0 term runs
inside the h1 AllReduce's ~35 us latency shadow.
"""
import sys

if '/opt/trn_rl_repo' not in sys.path:
    sys.path.insert(0, '/opt/trn_rl_repo')

import ml_dtypes
import numpy as np

import concourse.bass as bass
import concourse.bacc as bacc
import concourse.tile as tile
from concourse import mybir
from concourse.bass_utils import run_bass_kernel_spmd

HID = 128
BATCH = 4
SEQ = 2048
D_STATE = 16
D_CONV = 4
D_INNER = 256
DT_RANK = 8

L = SEQ
C = HID
CW = 512
NCH = L // CW
PW = 1024
NPW = L // PW
f32 = mybir.dt.float32
bf16 = mybir.dt.bfloat16
mult = mybir.AluOpType.mult
add = mybir.AluOpType.add
sub = mybir.AluOpType.subtract
AF = mybir.ActivationFunctionType

_PROGRAM = None


def _declare(nc):
    dpf = lambda name, shape: nc.declare_dram_parameter(name, list(shape), f32,
                                                        isOutput=False)
    dph = lambda name, shape: nc.declare_dram_parameter(name, list(shape), bf16,
                                                        isOutput=False)
    p = {}
    p['x'] = dpf('x', (C, L))
    for n in ('wlmT', 'wlgT', 'wcT', 'loT'):
        p[n] = dph(n, (C, C))
    for h in range(2):
        for k in range(D_CONV):
            p[f'wk{h}{k}'] = dph(f'wk{h}{k}', (C, C))
        p[f'inwzT{h}'] = dph(f'inwzT{h}', (C, C))
        p[f'owTA{h}'] = dph(f'owTA{h}', (128, C))
        p[f'owTB{h}'] = dph(f'owTB{h}', (128, C))
        p[f'xpwT{h}'] = dph(f'xpwT{h}', (128, DT_RANK + 2 * D_STATE))
        p[f'dtwT{h}'] = dph(f'dtwT{h}', (DT_RANK, 128))
    p['ident'] = dph('ident', (128, 128))
    for h in range(2):
        p[f'diagD{h}'] = dph(f'diagD{h}', (128, 128))
    p['avec'] = dpf('avec', (128, 32))
    for n in ('conv_b', 'dt_b'):
        p[n] = dpf(n, (128, 2))
    for n in ('bias_lm', 'bias_lg', 'bias_c', 'lo_b', 'ln_g', 'ln_b'):
        p[n] = dpf(n, (C, 1))
    p['y'] = nc.declare_dram_parameter('y', [C, L], f32, isOutput=True)
    return p


class B:
    pass


def _ln_stats_mm(b, x_mm, x_sub, out_bf):
    """LayerNorm over the 128 channels per column; stage-major to keep the
    ACT table set stable.  x_mm must be bf16 (fast-path PE matmuls); x_sub
    feeds the mean-subtract and may be f32 for precision."""
    nc = b.nc
    rows_bf = b.pb.tile([2, L], bf16, name='lnrowsb', tag='lnb')
    rows_f = b.pb.tile([1, L], f32, name='lnrowsf', tag='lnf')
    ex = rows_bf[0:1, :]
    rr = rows_f[0:1, :]
    nrm0 = b.pb.tile([C, L], bf16, name='nrm0', tag='xmf')
    sq2 = b.pb.tile([C, L], bf16, name='sq2', tag='y0')
    csl = [slice(ci * CW, (ci + 1) * CW) for ci in range(NCH)]
    for cs in csl:
        ps0 = b.ps.tile([1, CW], f32, name='bank', tag='bank')
        nc.tensor.matmul(ps0, b.ones_col, x_mm[:, cs], start=True, stop=True)
        nc.scalar.activation(ex[:, cs], ps0, AF.Identity, bias=0.0, scale=1.0 / C)
    for cs in csl:
        psb = b.ps.tile([128, CW], f32, name='bank', tag='bank')
        nc.tensor.matmul(psb, b.ones_row, ex[:, cs], start=True, stop=True)
        nc.vector.scalar_tensor_tensor(nrm0[:, cs], x_sub[:, cs], 1.0, psb,
                                       mult, sub)
    for cs in csl:
        nc.scalar.activation(sq2[:, cs], nrm0[:, cs], AF.Square)
    for cs in csl:
        psv = b.ps.tile([1, CW], f32, name='bank', tag='bank')
        nc.tensor.matmul(psv, b.ones_col, sq2[:, cs], start=True, stop=True)
        nc.scalar.activation(rr[:, cs], psv, AF.Ln, bias=b.eps_t[:, :],
                             scale=1.0 / C)
    for cs in csl:
        nc.scalar.activation(ex[:, cs], rr[:, cs], AF.Exp, bias=0.0, scale=-0.5)
    for cs in csl:
        psr = b.ps.tile([128, CW], f32, name='bank', tag='bank')
        nc.tensor.matmul(psr, b.ones_row, ex[:, cs], start=True, stop=True)
        nc.vector.scalar_tensor_tensor(out_bf[:, cs], nrm0[:, cs], 1.0, psr,
                                       mult, mult)


def _proj(b, lhsT, rhs, out, func, bias, rows=C, out_off=0):
    nc = b.nc
    for pi in range(NPW):
        ps = b.ps.tile([rows, PW], f32, name='bank', tag='bank')
        for half in range(2):
            cs = slice(pi * PW + half * CW, pi * PW + (half + 1) * CW)
            nc.tensor.matmul(ps[:, half * CW:(half + 1) * CW], lhsT, rhs[:, cs],
                             start=True, stop=True)
        ocs = slice(out_off + pi * PW, out_off + (pi + 1) * PW)
        nc.scalar.activation(out[:, ocs], ps, func, bias=bias)


def _build_body(nc, tc, p, ctx):
    b = B()
    b.nc = nc
    b.io = ctx.enter_context(tc.tile_pool(name='io', bufs=1))
    b.pb = ctx.enter_context(tc.tile_pool(name='pb', bufs=1))
    b.pf = ctx.enter_context(tc.tile_pool(name='pf', bufs=2))
    b.bc = ctx.enter_context(tc.tile_pool(name='bc', bufs=3))
    b.cb = ctx.enter_context(tc.tile_pool(name='cb', bufs=3))
    b.da = ctx.enter_context(tc.tile_pool(name='da', bufs=2))
    b.du = ctx.enter_context(tc.tile_pool(name='du', bufs=3))
    b.ht = ctx.enter_context(tc.tile_pool(name='ht', bufs=3))
    b.yc = ctx.enter_context(tc.tile_pool(name='yc', bufs=3))
    b.ps = ctx.enter_context(tc.tile_pool(name='ps', bufs=2, space='PSUM'))
    b.py = ctx.enter_context(tc.tile_pool(name='py', bufs=1, space='PSUM'))
    b.dram = ctx.enter_context(tc.tile_pool(name='drm', bufs=1, space='DRAM'))

    # LN-critical state first: the x DMA, the ones/eps memsets and the
    # input layernorm only need these — the ~30-deep weight DMA issue
    # stream on the sync queue then runs behind the LN instead of in
    # front of it.
    x = b.pf.tile([C, L], f32, name='x', tag='f')
    for ci in range(NCH):
        cs = slice(ci * CW, (ci + 1) * CW)
        nc.sync.dma_start(out=x[:, cs], in_=p['x'][:, cs])
    ones_col = b.io.tile([C, 1], bf16, name='ones_col', tag='ones_col')
    nc.vector.memset(ones_col, 1.0)
    b.ones_col = ones_col
    ones_row = b.io.tile([1, 128], bf16, name='ones_row', tag='ones_row')
    nc.vector.memset(ones_row, 1.0)
    b.ones_row = ones_row
    eps_t = b.io.tile([1, 1], f32, name='lneps', tag='lneps')
    nc.vector.memset(eps_t, 1e-5)
    b.eps_t = eps_t
    # bf16 copy of x for the fast-path (bf16 weights) LN matmuls
    xb = b.pb.tile([C, L], bf16, name='xb', tag='t2')
    for ci in range(NCH):
        cs = slice(ci * CW, (ci + 1) * CW)
        nc.scalar.activation(xb[:, cs], x[:, cs], AF.Identity, bias=0.0)

    # ---- P1: input layernorm (stats from bf16 x; subtract still f32 x) ----
    nrm = b.pb.tile([C, L], bf16, name='nrm', tag='nrmo')
    _ln_stats_mm(b, xb, x, nrm)

    W = {}
    wspec = [('wlmT', (C, C)), ('wlgT', (C, C)), ('wcT', (C, C)),
             ('loT', (C, C)), ('ident', (128, 128)),
             ('diagD0', (128, 128)), ('diagD1', (128, 128))]
    for h in range(2):
        wspec += [(f'wk{h}{k}', (C, C)) for k in range(D_CONV)]
        wspec += [(f'inwzT{h}', (C, C)), (f'owTA{h}', (128, C)),
                  (f'owTB{h}', (128, C)),
                  (f'xpwT{h}', (128, 40)), (f'dtwT{h}', (8, 128))]
    for n, shape in wspec:
        if n.startswith('dtwT'):
            W[n] = b.io.tile([40, shape[1]], bf16, name=n, tag=n)
            nc.sync.dma_start(out=W[n][32:40, :], in_=p[n][:, :])
            W[n] = W[n][32:40, :]
        else:
            W[n] = b.io.tile(list(shape), bf16, name=n, tag=n)
            nc.sync.dma_start(out=W[n], in_=p[n][:, :])
    V = {}
    V['avec'] = b.io.tile([128, 32], f32, name='avec', tag='avec')
    nc.sync.dma_start(out=V['avec'], in_=p['avec'][:, :])
    for n in ('conv_b', 'dt_b'):
        V[n] = b.io.tile([128, 2], f32, name=n, tag=n)
        nc.sync.dma_start(out=V[n], in_=p[n][:, :])
    for n in ('bias_lm', 'bias_lg', 'bias_c', 'lo_b', 'ln_g', 'ln_b'):
        V[n] = b.io.tile([C, 1], f32, name=n, tag=n)
        nc.sync.dma_start(out=V[n], in_=p[n][:, :])

    # ---- P2 ----
    xmf = b.pb.tile([C, L], bf16, name='xmf', tag='xmf')
    _proj(b, W['wlmT'], nrm, xmf, AF.Identity, V['bias_lm'][:, :])
    xm_pad = b.pb.tile([C, D_CONV - 1 + L], bf16, name='xm_pad', tag='xm_pad')
    nc.vector.memset(xm_pad[:, 0:D_CONV - 1], 0.0)
    _proj(b, W['wcT'], xmf, xm_pad, AF.Silu, V['bias_c'][:, :],
          out_off=D_CONV - 1)

    uc = []
    for h in range(2):
        uct = b.pb.tile([128, L], bf16, name=f'uc{h}', tag=f'uc{h}')
        for pi in range(NPW):
            psu = b.ps.tile([128, PW], f32, name='bank', tag='bank')
            for half in range(2):
                base = pi * PW + half * CW
                for k in range(D_CONV):
                    nc.tensor.matmul(psu[:, half * CW:(half + 1) * CW],
                                     W[f'wk{h}{k}'],
                                     xm_pad[:, k + base:k + base + CW],
                                     start=(k == 0), stop=(k == D_CONV - 1))
            nc.scalar.activation(uct[:, pi * PW:(pi + 1) * PW], psu, AF.Silu,
                                 bias=V['conv_b'][:, h:h + 1])
        uc.append(uct)

    # dbl rows: 0-15 B, 16-31 C, 32-39 dtr (xp_w rows reordered host-side)
    dbl_sb = b.pb.tile([40, L], bf16, name='dbl_sb', tag='dbl_sb')
    dtr = dbl_sb[32:40, :]
    for pi in range(NPW):
        psd = b.ps.tile([40, PW], f32, name='bank', tag='bank')
        for half in range(2):
            hs = slice(half * CW, (half + 1) * CW)
            cs = slice(pi * PW + half * CW, pi * PW + (half + 1) * CW)
            nc.tensor.matmul(psd[:, hs], W['xpwT0'], uc[0][:, cs],
                             start=True, stop=False)
            nc.tensor.matmul(psd[:, hs], W['xpwT1'], uc[1][:, cs],
                             start=False, stop=True)
        nc.scalar.activation(dbl_sb[:, pi * PW:(pi + 1) * PW], psd,
                             AF.Identity, bias=0.0)
    bc_d = b.dram.tile([32, L], bf16, name='bc_d', tag='bc_d')
    nc.sync.dma_start(out=bc_d, in_=dbl_sb[0:32, :])

    # dt = ln(1 + exp(dt_w @ dtr + dt_b)); f32 copy feeds the da exps,
    # bf16 copy feeds the 2x-mode dtu multiply
    dt = []
    dtu = []
    for h in range(2):
        z1 = b.pf.tile([128, L], f32, name=f'z1{h}', tag='f')
        _proj(b, W[f'dtwT{h}'], dtr, z1, AF.Exp, V['dt_b'][:, h:h + 1],
              rows=128)
        dtt = b.pb.tile([128, L], f32, name=f'dt{h}', tag=f'dt{h}')
        nc.scalar.activation(dtt, z1, AF.Ln, bias=1.0, scale=1.0)
        dtt_bf = b.pb.tile([128, L], bf16, name=f'dtb{h}', tag=f'dtb{h}')
        nc.scalar.activation(dtt_bf, z1, AF.Ln, bias=1.0, scale=1.0)
        dt.append(dtt)
        dtut = b.pb.tile([128, L], bf16, name=f'dtu{h}', tag=f'dtu{h}')
        nc.vector.tensor_tensor(dtut, dtt_bf, uc[h], mult)
        dtu.append(dtut)

    sz = []
    yz = []

    def scan_block(h):
        psy = b.py.tile([128, L], f32, name='psy', tag='psy')
        for s in range(D_STATE):
            j = 16 * h + s
            b_bc = b.bc.tile([128, L], bf16, name='b_bc', tag='b_bc')
            src = bass.AP(tensor=bc_d.tensor, offset=bc_d.offset + s * L,
                          ap=[[0, 128], [1, L]])
            nc.sync.dma_start(out=b_bc, in_=src)
            c_bc = b.cb.tile([128, L], bf16, name='c_bc', tag='c_bc')
            src = bass.AP(tensor=bc_d.tensor, offset=bc_d.offset + (16 + s) * L,
                          ap=[[0, 128], [1, L]])
            nc.sync.dma_start(out=c_bc, in_=src)

            da = b.da.tile([128, L], f32, name='da', tag='da')
            nc.scalar.activation(da, dt[h], AF.Exp, bias=0.0,
                                 scale=V['avec'][:, j:j + 1])
            dbu = b.du.tile([128, L], bf16, name='dbu', tag='dbu')
            nc.vector.tensor_tensor(dbu, dtu[h], b_bc, mult)
            ht = b.ht.tile([128, L], bf16, name='ht', tag='ht')
            nc.vector.tensor_tensor_scan(ht, da, dbu, 0.0, mult, add)
            ycm = b.yc.tile([128, L], bf16, name='ycm', tag='ycm')
            nc.vector.tensor_tensor(ycm, ht, c_bc, mult)
            for ci in range(NCH):
                cs = slice(ci * CW, (ci + 1) * CW)
                nc.tensor.matmul(psy[:, cs], W['ident'], ycm[:, cs],
                                 start=(s == 0), stop=False,
                                 skip_group_check=True)
        # fold uc * D into psy on PE (diagonal weights), closing the group
        for ci in range(NCH):
            cs = slice(ci * CW, (ci + 1) * CW)
            nc.tensor.matmul(psy[:, cs], W[f'diagD{h}'], uc[h][:, cs],
                             start=False, stop=True, skip_group_check=True)
        return psy

    def finish_half(h, psy):
        """psy -> yq -> yz -> out-projection -> staging -> AllReduce, all
        at PW granularity so the ACT/DVE/PE/DMA stages pipeline."""
        yq = b.pb.tile([128, L], bf16, name=f'yq{h}', tag=f'dtu{h}')
        yzt = b.pb.tile([128, L], bf16, name=f'yz{h}', tag=f'yz{h}')
        yz.append(yzt)
        for pi in range(NPW):
            pcs = slice(pi * PW, (pi + 1) * PW)
            nc.scalar.activation(yq[:, pcs], psy[:, pcs], AF.Identity,
                                 bias=0.0)
            nc.vector.tensor_tensor(yzt[:, pcs], yq[:, pcs], sz[h][:, pcs],
                                    mult)
        y_out = b.pb.tile([C, L], bf16, name=f'y{h}t', tag=f'y{h}t')
        cc_in = b.dram.tile([C, L], bf16, name=f'cc_in{h}', tag=f'cc_in{h}')
        for pi in range(NPW):
            ps = b.ps.tile([C, PW], f32, name='bank', tag='bank')
            for half in range(2):
                hs = slice(half * CW, (half + 1) * CW)
                a0 = pi * PW + half * CW
                a1 = pi * PW + (half + 1) * CW
                nc.tensor.matmul(ps[:, hs], W[f'owTA{h}'], yzt[:, a0:a1],
                                 start=True, stop=False)
                rcs = yzt[:, L - a1:L - a0][:, ::-1]
                nc.tensor.matmul(ps[:, hs], W[f'owTB{h}'], rcs,
                                 start=False, stop=True)
            ocs = slice(pi * PW, (pi + 1) * PW)
            nc.scalar.activation(y_out[:, ocs], ps, AF.Identity, bias=0.0)
            nc.sync.dma_start(out=cc_in[:, ocs], in_=y_out[:, ocs])
        cc_out = b.dram.tile([C, L], bf16, name=f'cc_out{h}', tag=f'cc_out{h}')
        nc.gpsimd.collective_compute(
            'AllReduce', add,
            replica_groups=[[0, 1], [2, 3], [4, 5], [6, 7]],
            ins=[cc_in.opt()], outs=[cc_out.opt()])
        return cc_out

    # h = 0 scans; gate/z projections run on ACT/PE meanwhile
    psy0 = scan_block(0)
    gate = b.pb.tile([C, L], bf16, name='gate', tag='gate')
    _proj(b, W['wlgT'], nrm, gate, AF.Silu, V['bias_lg'][:, :])
    for h in range(2):
        szt = b.pb.tile([128, L], bf16, name=f'sz{h}', tag=f'sz{h}')
        _proj(b, W[f'inwzT{h}'],
              xm_pad[:, D_CONV - 1:D_CONV - 1 + L], szt, AF.Silu, 0.0)
        sz.append(szt)

    # h=0 out-projection + its AllReduce, hidden under the h=1 scan block
    cc_out0 = finish_half(0, psy0)
    psy1 = scan_block(1)
    cc_out1 = finish_half(1, psy1)

    # ---- P5, split so the h0 part runs in AR1's latency shadow ----
    y_sum = b.pb.tile([C, L], bf16, name='y_sum', tag='y0')
    nc.sync.dma_start(out=y_sum, in_=cc_out0)
    g1a = b.pb.tile([C, L], bf16, name='g1a', tag='nrmo')
    nc.vector.tensor_tensor(g1a, y_sum, gate, mult)
    ps_t2 = []
    for pi in range(NPW):
        ps = b.ps.tile([C, PW], f32, name='bank', tag='bank')
        for half in range(2):
            cs = slice(pi * PW + half * CW, pi * PW + (half + 1) * CW)
            nc.tensor.matmul(ps[:, half * CW:(half + 1) * CW], W['loT'],
                             g1a[:, cs], start=True, stop=False,
                             skip_group_check=True)
        ps_t2.append(ps)
    ysum2 = b.pb.tile([C, L], bf16, name='ysum2', tag='ysum2')
    nc.sync.dma_start(out=ysum2, in_=cc_out1)
    g1b = b.pb.tile([C, L], bf16, name='g1b', tag='y1t')
    nc.vector.tensor_tensor(g1b, ysum2, gate, mult)
    t2 = b.pb.tile([C, L], bf16, name='t2', tag='t2')
    for pi in range(NPW):
        for half in range(2):
            cs = slice(pi * PW + half * CW, pi * PW + (half + 1) * CW)
            nc.tensor.matmul(ps_t2[pi][:, half * CW:(half + 1) * CW], W['loT'],
                             g1b[:, cs], start=False, stop=True,
                             skip_group_check=True)
        ocs = slice(pi * PW, (pi + 1) * PW)
        nc.scalar.activation(t2[:, ocs], ps_t2[pi], AF.Identity,
                             bias=V['lo_b'][:, :])

    # final LN chunk-major (2 PW chunks) so its stages pipeline with the
    # g1b/lo chain above, with per-chunk output scale + DMA
    rows_bf = b.pb.tile([2, L], bf16, name='lnrowsb2', tag='lnb')
    rows_f = b.pb.tile([1, L], f32, name='lnrowsf2', tag='lnf')
    ex = rows_bf[0:1, :]
    rr = rows_f[0:1, :]
    nrm0 = b.pb.tile([C, L], bf16, name='nrm0f', tag='xmf')
    sq2 = b.pb.tile([C, L], bf16, name='sq2f', tag='y0')
    o1 = b.pb.tile([C, L], bf16, name='o1', tag='xm_pad')
    out_sb = b.pf.tile([C, L], f32, name='out_sb', tag='f')
    for pi in range(NPW):
        csl = [slice(pi * PW + hh * CW, pi * PW + (hh + 1) * CW)
               for hh in range(2)]
        for cs in csl:
            ps0 = b.ps.tile([1, CW], f32, name='bank', tag='bank')
            nc.tensor.matmul(ps0, b.ones_col, t2[:, cs], start=True, stop=True)
            nc.scalar.activation(ex[:, cs], ps0, AF.Identity, bias=0.0,
                                 scale=1.0 / C)
        for cs in csl:
            psb = b.ps.tile([128, CW], f32, name='bank', tag='bank')
            nc.tensor.matmul(psb, b.ones_row, ex[:, cs], start=True, stop=True)
            nc.vector.scalar_tensor_tensor(nrm0[:, cs], t2[:, cs], 1.0, psb,
                                           mult, sub)
        for cs in csl:
            nc.scalar.activation(sq2[:, cs], nrm0[:, cs], AF.Square)
        for cs in csl:
            psv = b.ps.tile([1, CW], f32, name='bank', tag='bank')
            nc.tensor.matmul(psv, b.ones_col, sq2[:, cs], start=True, stop=True)
            nc.scalar.activation(rr[:, cs], psv, AF.Ln, bias=b.eps_t[:, :],
                                 scale=1.0 / C)
        for cs in csl:
            nc.scalar.activation(ex[:, cs], rr[:, cs], AF.Exp, bias=0.0,
                                 scale=-0.5)
        for cs in csl:
            psr = b.ps.tile([128, CW], f32, name='bank', tag='bank')
            nc.tensor.matmul(psr, b.ones_row, ex[:, cs], start=True, stop=True)
            nc.vector.scalar_tensor_tensor(o1[:, cs], nrm0[:, cs], 1.0, psr,
                                           mult, mult)
        pcs = slice(pi * PW, (pi + 1) * PW)
        nc.vector.tensor_scalar(out_sb[:, pcs], o1[:, pcs], V['ln_g'][:, :],
                                V['ln_b'][:, :], mult, add)
        nc.sync.dma_start(out=p['y'][:, pcs], in_=out_sb[:, pcs])


def _build_program():
    import contextlib
    nc = bacc.Bacc('TRN2', target_bir_lowering=False, debug=False, num_devices=8)
    p = _declare(nc)
    with tile.TileContext(nc) as tc:
        with contextlib.ExitStack() as ctx:
            _build_body(nc, tc, p, ctx)
    nc.compile()
    return nc


def _prep_core_inputs(inputs, bidx, d):
    g = lambda n: np.asarray(inputs[n], dtype=np.float32)
    x = g('x')
    ln_g = g('ln_g')
    ln_b = g('ln_b')
    pre = 'mf_' if d == 0 else 'mb_'
    P = lambda n: np.asarray(inputs[pre + n], dtype=np.float32)

    lm_w, lm_b = g('lm_w'), g('lm_b')
    lg_w, lg_b = g('lg_w'), g('lg_b')
    lo_w, lo_b = g('lo_w'), g('lo_b')
    if d == 0:
        wc, cb = g('cf_w'), g('cf_b')
    else:
        wc, cb = np.ascontiguousarray(g('cb_w')[:, ::-1]), g('cb_b')

    A = -np.exp(P('Alog'))
    avec = np.zeros((128, 32), np.float32)
    for h in range(2):
        for s in range(16):
            avec[:, 16 * h + s] = A[128 * h:128 * (h + 1), s]

    bf = lambda a: np.ascontiguousarray(np.asarray(a, dtype=ml_dtypes.bfloat16))
    col = lambda v: np.ascontiguousarray(v.astype(np.float32).reshape(-1, 1))
    halves = lambda v: np.ascontiguousarray(
        np.stack([v[:128], v[128:]], axis=1).astype(np.float32))
    T = lambda w: np.ascontiguousarray(w.T)

    in_w = P('in_w')
    conv_w = P('conv_w')
    xpw = P('xp_w')
    xpw = np.concatenate([xpw[DT_RANK:], xpw[:DT_RANK]], axis=0)
    xpwT = np.ascontiguousarray(xpw.T)
    outwT = np.ascontiguousarray(P('out_w').T)
    dtwT = np.ascontiguousarray(P('dt_w').T)

    out = {
        'x': np.ascontiguousarray(x[bidx]),
        'wlmT': bf(T(lm_w * ln_g[None, :])),
        'wlgT': bf(T(lg_w * ln_g[None, :])),
        'wcT': bf(T(wc)),
        'loT': bf(T(lo_w)),
        'ident': bf(np.eye(128, dtype=np.float32)),
        'avec': avec,
        'conv_b': halves(P('conv_b')),
        'dt_b': halves(P('dt_b')),
        'bias_lm': col(lm_w @ ln_b + lm_b),
        'bias_lg': col(lg_w @ ln_b + lg_b),
        'bias_c': col(cb),
        'lo_b': col(lo_b),
        'ln_g': col(ln_g),
        'ln_b': col(ln_b),
    }
    for h in range(2):
        hsl = slice(128 * h, 128 * (h + 1))
        out[f'diagD{h}'] = bf(np.diag(P('D')[hsl]).astype(np.float32))
        for k in range(D_CONV):
            wk = in_w[hsl, :] * conv_w[hsl, k:k + 1]
            out[f'wk{h}{k}'] = bf(T(wk))
        out[f'inwzT{h}'] = bf(T(P('in_w')[256:][hsl, :]))
        ow = outwT[hsl, :]
        out[f'owTA{h}'] = bf(ow if d == 0 else np.zeros_like(ow))
        out[f'owTB{h}'] = bf(np.zeros_like(ow) if d == 0 else ow)
        out[f'xpwT{h}'] = bf(xpwT[hsl, :])
        out[f'dtwT{h}'] = bf(dtwT[:, hsl])
    return out


def get_program():
    global _PROGRAM
    if _PROGRAM is None:
        _PROGRAM = _build_program()
    return _PROGRAM


def run(inputs, **run_kwargs):
    nc = get_program()
    in_maps = [_prep_core_inputs(inputs, c // 2, c % 2) for c in range(8)]
    res = run_bass_kernel_spmd(nc, in_maps, core_ids=list(range(8)), **run_kwargs)
    out = np.stack([res.results[2 * b]['y'] for b in range(BATCH)], axis=0)
    return out, res


def kernel(**inputs) -> np.ndarray:
    out, _ = run(inputs)
    return out.astype(np.float32)


# revision 36
# speedup vs baseline: 1.1121x; 1.0173x over previous
"""Bidirectional Mamba block (BiT_MamSleep) on 8 TRN2 NeuronCores — v10.

Sharding: core c handles (batch b = c//2, direction dir = c%2); pairwise
AllReduce joins the two directions; both cores compute the tail redundantly.

s-major scan layout: 32 tiles of [128 part = d (one half of d_inner),
free = t], one per (half h, state s).  dA_s comes straight from ACT exp with
per-partition scale A[:, s]; B/C are row-broadcast per state; the sum over s
is identity-matmul PSUM accumulation on PE.  The depthwise conv is folded
into the in-projection (4 shifted-AP matmuls).  All matmuls bf16.

Scan-phase elementwise ops are bf16 tensor_tensor (the only DVE op family
with a 2x perf-mode uop; scalar_tensor_tensor and the scan itself run 1x).
uc*D rides the psy PSUM accumulation as a diagonal matmul.  The h0
AllReduce hides under the h1 scan; the lo-projection of the h0 term runs
inside the h1 AllReduce's ~35 us latency shadow.
"""
import sys

if '/opt/trn_rl_repo' not in sys.path:
    sys.path.insert(0, '/opt/trn_rl_repo')

import ml_dtypes
import numpy as np

import concourse.bass as bass
import concourse.bacc as bacc
import concourse.tile as tile
from concourse import mybir
from concourse.bass_utils import run_bass_kernel_spmd

HID = 128
BATCH = 4
SEQ = 2048
D_STATE = 16
D_CONV = 4
D_INNER = 256
DT_RANK = 8

L = SEQ
C = HID
CW = 512
NCH = L // CW
PW = 1024
NPW = L // PW
f32 = mybir.dt.float32
bf16 = mybir.dt.bfloat16
mult = mybir.AluOpType.mult
add = mybir.AluOpType.add
sub = mybir.AluOpType.subtract
AF = mybir.ActivationFunctionType

_PROGRAM = None


def _declare(nc):
    dpf = lambda name, shape: nc.declare_dram_parameter(name, list(shape), f32,
                                                        isOutput=False)
    dph = lambda name, shape: nc.declare_dram_parameter(name, list(shape), bf16,
                                                        isOutput=False)
    p = {}
    p['x'] = dpf('x', (C, L))
    for n in ('wlmT', 'wlgT', 'wcT', 'loT'):
        p[n] = dph(n, (C, C))
    for h in range(2):
        for k in range(D_CONV):
            p[f'wk{h}{k}'] = dph(f'wk{h}{k}', (C, C))
        p[f'inwzT{h}'] = dph(f'inwzT{h}', (C, C))
        p[f'owTA{h}'] = dph(f'owTA{h}', (128, C))
        p[f'owTB{h}'] = dph(f'owTB{h}', (128, C))
        p[f'xpwT{h}'] = dph(f'xpwT{h}', (128, DT_RANK + 2 * D_STATE))
        p[f'dtwT{h}'] = dph(f'dtwT{h}', (DT_RANK, 128))
    p['ident'] = dph('ident', (128, 128))
    for h in range(2):
        p[f'diagD{h}'] = dph(f'diagD{h}', (128, 128))
    p['avec'] = dpf('avec', (128, 32))
    for n in ('conv_b', 'dt_b'):
        p[n] = dpf(n, (128, 2))
    for n in ('bias_lm', 'bias_lg', 'bias_c', 'lo_b', 'ln_g', 'ln_b'):
        p[n] = dpf(n, (C, 1))
    p['y'] = nc.declare_dram_parameter('y', [C, L], f32, isOutput=True)
    return p


class B:
    pass


def _ln_stats_mm(b, x_mm, x_sub, out_bf):
    """LayerNorm over the 128 channels per column; stage-major to keep the
    ACT table set stable.  x_mm must be bf16 (fast-path PE matmuls); x_sub
    feeds the mean-subtract and may be f32 for precision."""
    nc = b.nc
    rows_bf = b.pb.tile([2, L], bf16, name='lnrowsb', tag='lnb')
    rows_f = b.pb.tile([1, L], f32, name='lnrowsf', tag='lnf')
    ex = rows_bf[0:1, :]
    rr = rows_f[0:1, :]
    nrm0 = b.pb.tile([C, L], bf16, name='nrm0', tag='xmf')
    sq2 = b.pb.tile([C, L], bf16, name='sq2', tag='y0')
    csl = [slice(ci * CW, (ci + 1) * CW) for ci in range(NCH)]
    for cs in csl:
        ps0 = b.ps.tile([1, CW], f32, name='bank', tag='bank')
        nc.tensor.matmul(ps0, b.ones_col, x_mm[:, cs], start=True, stop=True)
        nc.scalar.activation(ex[:, cs], ps0, AF.Identity, bias=0.0, scale=1.0 / C)
    for cs in csl:
        psb = b.ps.tile([128, CW], f32, name='bank', tag='bank')
        nc.tensor.matmul(psb, b.ones_row, ex[:, cs], start=True, stop=True)
        nc.vector.scalar_tensor_tensor(nrm0[:, cs], x_sub[:, cs], 1.0, psb,
                                       mult, sub)
    for cs in csl:
        nc.scalar.activation(sq2[:, cs], nrm0[:, cs], AF.Square)
    for cs in csl:
        psv = b.ps.tile([1, CW], f32, name='bank', tag='bank')
        nc.tensor.matmul(psv, b.ones_col, sq2[:, cs], start=True, stop=True)
        nc.scalar.activation(rr[:, cs], psv, AF.Ln, bias=b.eps_t[:, :],
                             scale=1.0 / C)
    for cs in csl:
        nc.scalar.activation(ex[:, cs], rr[:, cs], AF.Exp, bias=0.0, scale=-0.5)
    for cs in csl:
        psr = b.ps.tile([128, CW], f32, name='bank', tag='bank')
        nc.tensor.matmul(psr, b.ones_row, ex[:, cs], start=True, stop=True)
        nc.vector.scalar_tensor_tensor(out_bf[:, cs], nrm0[:, cs], 1.0, psr,
                                       mult, mult)


def _proj(b, lhsT, rhs, out, func, bias, rows=C, out_off=0):
    nc = b.nc
    for pi in range(NPW):
        ps = b.ps.tile([rows, PW], f32, name='bank', tag='bank')
        for half in range(2):
            cs = slice(pi * PW + half * CW, pi * PW + (half + 1) * CW)
            nc.tensor.matmul(ps[:, half * CW:(half + 1) * CW], lhsT, rhs[:, cs],
                             start=True, stop=True)
        ocs = slice(out_off + pi * PW, out_off + (pi + 1) * PW)
        nc.scalar.activation(out[:, ocs], ps, func, bias=bias)


def _build_body(nc, tc, p, ctx):
    b = B()
    b.nc = nc
    b.io = ctx.enter_context(tc.tile_pool(name='io', bufs=1))
    b.pb = ctx.enter_context(tc.tile_pool(name='pb', bufs=1))
    b.pf = ctx.enter_context(tc.tile_pool(name='pf', bufs=2))
    b.bc = ctx.enter_context(tc.tile_pool(name='bc', bufs=3))
    b.cb = ctx.enter_context(tc.tile_pool(name='cb', bufs=3))
    b.da = ctx.enter_context(tc.tile_pool(name='da', bufs=2))
    b.du = ctx.enter_context(tc.tile_pool(name='du', bufs=3))
    b.ht = ctx.enter_context(tc.tile_pool(name='ht', bufs=3))
    b.yc = ctx.enter_context(tc.tile_pool(name='yc', bufs=3))
    b.ps = ctx.enter_context(tc.tile_pool(name='ps', bufs=2, space='PSUM'))
    b.py = ctx.enter_context(tc.tile_pool(name='py', bufs=1, space='PSUM'))
    b.dram = ctx.enter_context(tc.tile_pool(name='drm', bufs=1, space='DRAM'))

    # LN-critical state first: the x DMA, the ones/eps memsets and the
    # input layernorm only need these — the ~30-deep weight DMA issue
    # stream on the sync queue then runs behind the LN instead of in
    # front of it.
    x = b.pf.tile([C, L], f32, name='x', tag='f')
    for ci in range(NCH):
        cs = slice(ci * CW, (ci + 1) * CW)
        nc.sync.dma_start(out=x[:, cs], in_=p['x'][:, cs])
    ones_col = b.io.tile([C, 1], bf16, name='ones_col', tag='ones_col')
    nc.vector.memset(ones_col, 1.0)
    b.ones_col = ones_col
    ones_row = b.io.tile([1, 128], bf16, name='ones_row', tag='ones_row')
    nc.vector.memset(ones_row, 1.0)
    b.ones_row = ones_row
    eps_t = b.io.tile([1, 1], f32, name='lneps', tag='lneps')
    nc.vector.memset(eps_t, 1e-5)
    b.eps_t = eps_t
    # bf16 copy of x for the fast-path (bf16 weights) LN matmuls
    xb = b.pb.tile([C, L], bf16, name='xb', tag='t2')
    for ci in range(NCH):
        cs = slice(ci * CW, (ci + 1) * CW)
        nc.vector.tensor_copy(xb[:, cs], x[:, cs])

    # ---- P1: input layernorm (stats from bf16 x; subtract still f32 x) ----
    nrm = b.pb.tile([C, L], bf16, name='nrm', tag='nrmo')
    _ln_stats_mm(b, xb, x, nrm)

    W = {}
    wspec = [('wlmT', (C, C)), ('wlgT', (C, C)), ('wcT', (C, C)),
             ('loT', (C, C)), ('ident', (128, 128)),
             ('diagD0', (128, 128)), ('diagD1', (128, 128))]
    for h in range(2):
        wspec += [(f'wk{h}{k}', (C, C)) for k in range(D_CONV)]
        wspec += [(f'inwzT{h}', (C, C)), (f'owTA{h}', (128, C)),
                  (f'owTB{h}', (128, C)),
                  (f'xpwT{h}', (128, 40)), (f'dtwT{h}', (8, 128))]
    for n, shape in wspec:
        if n.startswith('dtwT'):
            W[n] = b.io.tile([40, shape[1]], bf16, name=n, tag=n)
            nc.sync.dma_start(out=W[n][32:40, :], in_=p[n][:, :])
            W[n] = W[n][32:40, :]
        else:
            W[n] = b.io.tile(list(shape), bf16, name=n, tag=n)
            nc.sync.dma_start(out=W[n], in_=p[n][:, :])
    V = {}
    V['avec'] = b.io.tile([128, 32], f32, name='avec', tag='avec')
    nc.sync.dma_start(out=V['avec'], in_=p['avec'][:, :])
    for n in ('conv_b', 'dt_b'):
        V[n] = b.io.tile([128, 2], f32, name=n, tag=n)
        nc.sync.dma_start(out=V[n], in_=p[n][:, :])
    for n in ('bias_lm', 'bias_lg', 'bias_c', 'lo_b', 'ln_g', 'ln_b'):
        V[n] = b.io.tile([C, 1], f32, name=n, tag=n)
        nc.sync.dma_start(out=V[n], in_=p[n][:, :])

    # ---- P2 ----
    xmf = b.pb.tile([C, L], bf16, name='xmf', tag='xmf')
    _proj(b, W['wlmT'], nrm, xmf, AF.Identity, V['bias_lm'][:, :])
    xm_pad = b.pb.tile([C, D_CONV - 1 + L], bf16, name='xm_pad', tag='xm_pad')
    nc.vector.memset(xm_pad[:, 0:D_CONV - 1], 0.0)
    _proj(b, W['wcT'], xmf, xm_pad, AF.Silu, V['bias_c'][:, :],
          out_off=D_CONV - 1)

    uc = []
    for h in range(2):
        uct = b.pb.tile([128, L], bf16, name=f'uc{h}', tag=f'uc{h}')
        for pi in range(NPW):
            psu = b.ps.tile([128, PW], f32, name='bank', tag='bank')
            for half in range(2):
                base = pi * PW + half * CW
                for k in range(D_CONV):
                    nc.tensor.matmul(psu[:, half * CW:(half + 1) * CW],
                                     W[f'wk{h}{k}'],
                                     xm_pad[:, k + base:k + base + CW],
                                     start=(k == 0), stop=(k == D_CONV - 1))
            nc.scalar.activation(uct[:, pi * PW:(pi + 1) * PW], psu, AF.Silu,
                                 bias=V['conv_b'][:, h:h + 1])
        uc.append(uct)

    # dbl rows: 0-15 B, 16-31 C, 32-39 dtr (xp_w rows reordered host-side)
    dbl_sb = b.pb.tile([40, L], bf16, name='dbl_sb', tag='dbl_sb')
    dtr = dbl_sb[32:40, :]
    for pi in range(NPW):
        psd = b.ps.tile([40, PW], f32, name='bank', tag='bank')
        for half in range(2):
            hs = slice(half * CW, (half + 1) * CW)
            cs = slice(pi * PW + half * CW, pi * PW + (half + 1) * CW)
            nc.tensor.matmul(psd[:, hs], W['xpwT0'], uc[0][:, cs],
                             start=True, stop=False)
            nc.tensor.matmul(psd[:, hs], W['xpwT1'], uc[1][:, cs],
                             start=False, stop=True)
        nc.scalar.activation(dbl_sb[:, pi * PW:(pi + 1) * PW], psd,
                             AF.Identity, bias=0.0)
    bc_d = b.dram.tile([32, L], bf16, name='bc_d', tag='bc_d')
    nc.sync.dma_start(out=bc_d, in_=dbl_sb[0:32, :])

    # dt = ln(1 + exp(dt_w @ dtr + dt_b)); f32 copy feeds the da exps,
    # bf16 copy feeds the 2x-mode dtu multiply
    dt = []
    dtu = []
    for h in range(2):
        z1 = b.pf.tile([128, L], f32, name=f'z1{h}', tag='f')
        _proj(b, W[f'dtwT{h}'], dtr, z1, AF.Exp, V['dt_b'][:, h:h + 1],
              rows=128)
        dtt = b.pb.tile([128, L], f32, name=f'dt{h}', tag=f'dt{h}')
        nc.scalar.activation(dtt, z1, AF.Ln, bias=1.0, scale=1.0)
        dtt_bf = b.pb.tile([128, L], bf16, name=f'dtb{h}', tag=f'dtb{h}')
        nc.scalar.activation(dtt_bf, z1, AF.Ln, bias=1.0, scale=1.0)
        dt.append(dtt)
        dtut = b.pb.tile([128, L], bf16, name=f'dtu{h}', tag=f'dtu{h}')
        nc.vector.tensor_tensor(dtut, dtt_bf, uc[h], mult)
        dtu.append(dtut)

    sz = []
    yz = []

    def scan_block(h):
        psy = b.py.tile([128, L], f32, name='psy', tag='psy')
        for s in range(D_STATE):
            j = 16 * h + s
            b_bc = b.bc.tile([128, L], bf16, name='b_bc', tag='b_bc')
            src = bass.AP(tensor=bc_d.tensor, offset=bc_d.offset + s * L,
                          ap=[[0, 128], [1, L]])
            nc.sync.dma_start(out=b_bc, in_=src)
            c_bc = b.cb.tile([128, L], bf16, name='c_bc', tag='c_bc')
            src = bass.AP(tensor=bc_d.tensor, offset=bc_d.offset + (16 + s) * L,
                          ap=[[0, 128], [1, L]])
            nc.sync.dma_start(out=c_bc, in_=src)

            da = b.da.tile([128, L], f32, name='da', tag='da')
            nc.scalar.activation(da, dt[h], AF.Exp, bias=0.0,
                                 scale=V['avec'][:, j:j + 1])
            dbu = b.du.tile([128, L], bf16, name='dbu', tag='dbu')
            nc.vector.tensor_tensor(dbu, dtu[h], b_bc, mult)
            ht = b.ht.tile([128, L], bf16, name='ht', tag='ht')
            nc.vector.tensor_tensor_scan(ht, da, dbu, 0.0, mult, add)
            ycm = b.yc.tile([128, L], bf16, name='ycm', tag='ycm')
            nc.vector.tensor_tensor(ycm, ht, c_bc, mult)
            for ci in range(NCH):
                cs = slice(ci * CW, (ci + 1) * CW)
                nc.tensor.matmul(psy[:, cs], W['ident'], ycm[:, cs],
                                 start=(s == 0), stop=False,
                                 skip_group_check=True)
        # fold uc * D into psy on PE (diagonal weights), closing the group
        for ci in range(NCH):
            cs = slice(ci * CW, (ci + 1) * CW)
            nc.tensor.matmul(psy[:, cs], W[f'diagD{h}'], uc[h][:, cs],
                             start=False, stop=True, skip_group_check=True)
        return psy

    def finish_half(h, psy):
        """psy -> yq -> yz -> out-projection -> staging -> AllReduce, all
        at PW granularity so the ACT/DVE/PE/DMA stages pipeline."""
        yq = b.pb.tile([128, L], bf16, name=f'yq{h}', tag=f'dtu{h}')
        yzt = b.pb.tile([128, L], bf16, name=f'yz{h}', tag=f'yz{h}')
        yz.append(yzt)
        for pi in range(NPW):
            pcs = slice(pi * PW, (pi + 1) * PW)
            nc.scalar.activation(yq[:, pcs], psy[:, pcs], AF.Identity,
                                 bias=0.0)
            nc.vector.tensor_tensor(yzt[:, pcs], yq[:, pcs], sz[h][:, pcs],
                                    mult)
        y_out = b.pb.tile([C, L], bf16, name=f'y{h}t', tag=f'y{h}t')
        cc_in = b.dram.tile([C, L], bf16, name=f'cc_in{h}', tag=f'cc_in{h}')
        for pi in range(NPW):
            ps = b.ps.tile([C, PW], f32, name='bank', tag='bank')
            for half in range(2):
                hs = slice(half * CW, (half + 1) * CW)
                a0 = pi * PW + half * CW
                a1 = pi * PW + (half + 1) * CW
                nc.tensor.matmul(ps[:, hs], W[f'owTA{h}'], yzt[:, a0:a1],
                                 start=True, stop=False)
                rcs = yzt[:, L - a1:L - a0][:, ::-1]
                nc.tensor.matmul(ps[:, hs], W[f'owTB{h}'], rcs,
                                 start=False, stop=True)
            ocs = slice(pi * PW, (pi + 1) * PW)
            nc.scalar.activation(y_out[:, ocs], ps, AF.Identity, bias=0.0)
            nc.sync.dma_start(out=cc_in[:, ocs], in_=y_out[:, ocs])
        cc_out = b.dram.tile([C, L], bf16, name=f'cc_out{h}', tag=f'cc_out{h}')
        nc.gpsimd.collective_compute(
            'AllReduce', add,
            replica_groups=[[0, 1], [2, 3], [4, 5], [6, 7]],
            ins=[cc_in.opt()], outs=[cc_out.opt()])
        return cc_out

    # h = 0 scans; gate/z projections run on ACT/PE meanwhile
    psy0 = scan_block(0)
    gate = b.pb.tile([C, L], bf16, name='gate', tag='gate')
    _proj(b, W['wlgT'], nrm, gate, AF.Silu, V['bias_lg'][:, :])
    for h in range(2):
        szt = b.pb.tile([128, L], bf16, name=f'sz{h}', tag=f'sz{h}')
        _proj(b, W[f'inwzT{h}'],
              xm_pad[:, D_CONV - 1:D_CONV - 1 + L], szt, AF.Silu, 0.0)
        sz.append(szt)

    # h=0 out-projection + its AllReduce, hidden under the h=1 scan block
    cc_out0 = finish_half(0, psy0)
    psy1 = scan_block(1)
    cc_out1 = finish_half(1, psy1)

    # ---- P5, split so the h0 part runs in AR1's latency shadow ----
    y_sum = b.pb.tile([C, L], bf16, name='y_sum', tag='y0')
    nc.sync.dma_start(out=y_sum, in_=cc_out0)
    g1a = b.pb.tile([C, L], bf16, name='g1a', tag='nrmo')
    nc.vector.tensor_tensor(g1a, y_sum, gate, mult)
    ps_t2 = []
    for pi in range(NPW):
        ps = b.ps.tile([C, PW], f32, name='bank', tag='bank')
        for half in range(2):
            cs = slice(pi * PW + half * CW, pi * PW + (half + 1) * CW)
            nc.tensor.matmul(ps[:, half * CW:(half + 1) * CW], W['loT'],
                             g1a[:, cs], start=True, stop=False,
                             skip_group_check=True)
        ps_t2.append(ps)
    ysum2 = b.pb.tile([C, L], bf16, name='ysum2', tag='ysum2')
    nc.sync.dma_start(out=ysum2, in_=cc_out1)
    g1b = b.pb.tile([C, L], bf16, name='g1b', tag='y1t')
    nc.vector.tensor_tensor(g1b, ysum2, gate, mult)
    t2 = b.pb.tile([C, L], bf16, name='t2', tag='t2')
    for pi in range(NPW):
        for half in range(2):
            cs = slice(pi * PW + half * CW, pi * PW + (half + 1) * CW)
            nc.tensor.matmul(ps_t2[pi][:, half * CW:(half + 1) * CW], W['loT'],
                             g1b[:, cs], start=False, stop=True,
                             skip_group_check=True)
        ocs = slice(pi * PW, (pi + 1) * PW)
        nc.scalar.activation(t2[:, ocs], ps_t2[pi], AF.Identity,
                             bias=V['lo_b'][:, :])

    # final LN chunk-major (2 PW chunks) so its stages pipeline with the
    # g1b/lo chain above, with per-chunk output scale + DMA
    rows_bf = b.pb.tile([2, L], bf16, name='lnrowsb2', tag='lnb')
    rows_f = b.pb.tile([1, L], f32, name='lnrowsf2', tag='lnf')
    ex = rows_bf[0:1, :]
    rr = rows_f[0:1, :]
    nrm0 = b.pb.tile([C, L], bf16, name='nrm0f', tag='xmf')
    sq2 = b.pb.tile([C, L], bf16, name='sq2f', tag='y0')
    o1 = b.pb.tile([C, L], bf16, name='o1', tag='xm_pad')
    out_sb = b.pf.tile([C, L], f32, name='out_sb', tag='f')
    for pi in range(NPW):
        csl = [slice(pi * PW + hh * CW, pi * PW + (hh + 1) * CW)
               for hh in range(2)]
        for cs in csl:
            ps0 = b.ps.tile([1, CW], f32, name='bank', tag='bank')
            nc.tensor.matmul(ps0, b.ones_col, t2[:, cs], start=True, stop=True)
            nc.scalar.activation(ex[:, cs], ps0, AF.Identity, bias=0.0,
                                 scale=1.0 / C)
        for cs in csl:
            psb = b.ps.tile([128, CW], f32, name='bank', tag='bank')
            nc.tensor.matmul(psb, b.ones_row, ex[:, cs], start=True, stop=True)
            nc.vector.scalar_tensor_tensor(nrm0[:, cs], t2[:, cs], 1.0, psb,
                                           mult, sub)
        for cs in csl:
            nc.scalar.activation(sq2[:, cs], nrm0[:, cs], AF.Square)
        for cs in csl:
            psv = b.ps.tile([1, CW], f32, name='bank', tag='bank')
            nc.tensor.matmul(psv, b.ones_col, sq2[:, cs], start=True, stop=True)
            nc.scalar.activation(rr[:, cs], psv, AF.Ln, bias=b.eps_t[:, :],
                                 scale=1.0 / C)
        for cs in csl:
            nc.scalar.activation(ex[:, cs], rr[:, cs], AF.Exp, bias=0.0,
                                 scale=-0.5)
        for cs in csl:
            psr = b.ps.tile([128, CW], f32, name='bank', tag='bank')
            nc.tensor.matmul(psr, b.ones_row, ex[:, cs], start=True, stop=True)
            nc.vector.scalar_tensor_tensor(o1[:, cs], nrm0[:, cs], 1.0, psr,
                                           mult, mult)
        pcs = slice(pi * PW, (pi + 1) * PW)
        nc.vector.tensor_scalar(out_sb[:, pcs], o1[:, pcs], V['ln_g'][:, :],
                                V['ln_b'][:, :], mult, add)
        nc.sync.dma_start(out=p['y'][:, pcs], in_=out_sb[:, pcs])


def _build_program():
    import contextlib
    nc = bacc.Bacc('TRN2', target_bir_lowering=False, debug=False, num_devices=8)
    p = _declare(nc)
    with tile.TileContext(nc) as tc:
        with contextlib.ExitStack() as ctx:
            _build_body(nc, tc, p, ctx)
    nc.compile()
    return nc


def _prep_core_inputs(inputs, bidx, d):
    g = lambda n: np.asarray(inputs[n], dtype=np.float32)
    x = g('x')
    ln_g = g('ln_g')
    ln_b = g('ln_b')
    pre = 'mf_' if d == 0 else 'mb_'
    P = lambda n: np.asarray(inputs[pre + n], dtype=np.float32)

    lm_w, lm_b = g('lm_w'), g('lm_b')
    lg_w, lg_b = g('lg_w'), g('lg_b')
    lo_w, lo_b = g('lo_w'), g('lo_b')
    if d == 0:
        wc, cb = g('cf_w'), g('cf_b')
    else:
        wc, cb = np.ascontiguousarray(g('cb_w')[:, ::-1]), g('cb_b')

    A = -np.exp(P('Alog'))
    avec = np.zeros((128, 32), np.float32)
    for h in range(2):
        for s in range(16):
            avec[:, 16 * h + s] = A[128 * h:128 * (h + 1), s]

    bf = lambda a: np.ascontiguousarray(np.asarray(a, dtype=ml_dtypes.bfloat16))
    col = lambda v: np.ascontiguousarray(v.astype(np.float32).reshape(-1, 1))
    halves = lambda v: np.ascontiguousarray(
        np.stack([v[:128], v[128:]], axis=1).astype(np.float32))
    T = lambda w: np.ascontiguousarray(w.T)

    in_w = P('in_w')
    conv_w = P('conv_w')
    xpw = P('xp_w')
    xpw = np.concatenate([xpw[DT_RANK:], xpw[:DT_RANK]], axis=0)
    xpwT = np.ascontiguousarray(xpw.T)
    outwT = np.ascontiguousarray(P('out_w').T)
    dtwT = np.ascontiguousarray(P('dt_w').T)

    out = {
        'x': np.ascontiguousarray(x[bidx]),
        'wlmT': bf(T(lm_w * ln_g[None, :])),
        'wlgT': bf(T(lg_w * ln_g[None, :])),
        'wcT': bf(T(wc)),
        'loT': bf(T(lo_w)),
        'ident': bf(np.eye(128, dtype=np.float32)),
        'avec': avec,
        'conv_b': halves(P('conv_b')),
        'dt_b': halves(P('dt_b')),
        'bias_lm': col(lm_w @ ln_b + lm_b),
        'bias_lg': col(lg_w @ ln_b + lg_b),
        'bias_c': col(cb),
        'lo_b': col(lo_b),
        'ln_g': col(ln_g),
        'ln_b': col(ln_b),
    }
    for h in range(2):
        hsl = slice(128 * h, 128 * (h + 1))
        out[f'diagD{h}'] = bf(np.diag(P('D')[hsl]).astype(np.float32))
        for k in range(D_CONV):
            wk = in_w[hsl, :] * conv_w[hsl, k:k + 1]
            out[f'wk{h}{k}'] = bf(T(wk))
        out[f'inwzT{h}'] = bf(T(P('in_w')[256:][hsl, :]))
        ow = outwT[hsl, :]
        out[f'owTA{h}'] = bf(ow if d == 0 else np.zeros_like(ow))
        out[f'owTB{h}'] = bf(np.zeros_like(ow) if d == 0 else ow)
        out[f'xpwT{h}'] = bf(xpwT[hsl, :])
        out[f'dtwT{h}'] = bf(dtwT[:, hsl])
    return out


def get_program():
    global _PROGRAM
    if _PROGRAM is None:
        _PROGRAM = _build_program()
    return _PROGRAM


def run(inputs, **run_kwargs):
    nc = get_program()
    in_maps = [_prep_core_inputs(inputs, c // 2, c % 2) for c in range(8)]
    res = run_bass_kernel_spmd(nc, in_maps, core_ids=list(range(8)), **run_kwargs)
    out = np.stack([res.results[2 * b]['y'] for b in range(BATCH)], axis=0)
    return out, res


def kernel(**inputs) -> np.ndarray:
    out, _ = run(inputs)
    return out.astype(np.float32)
